# revision 4
# baseline (speedup 1.0000x reference)
"""GATv2 3-layer encoder on 8 Trainium2 NeuronCores (Bass/Tile).

Sharding: nodes are split contiguously across the 8 cores (graph parallel).
Edges (with self-loops) are owned by the destination node's core and sorted
by dst. Each layer: local matmuls (HL = h @ Wl, HR = h @ Wr), AllGather of
the HL shard, then a per-edge phase that gathers hl[src] / hr[dst] via
indirect DMA, computes GATv2 attention scores, and accumulates the per-dst
segment softmax + weighted feature sum with PE matmuls against an on-chip
generated 0/1 selector matrix.

kernel(**inputs) takes the FULL inputs and returns the FULL [100000, 64]
float32 output.
"""

import math
import os
from contextlib import ExitStack

import numpy as np

import concourse.bass as bass
import concourse.tile as tile
from concourse import bacc, mybir
from concourse.bass_utils import run_bass_kernel_spmd

# -------- problem config (hardcoded; must match reference.setup_inputs) ----
N_NODES = 100_000
N_EDGES = 1_600_000
NCORES = 8
NEG_SLOPE = 0.2
P = 128
# layer dims: (D_in, D_out, heads)
LAYERS = [(128, 128, 4), (128, 128, 4), (128, 64, 1)]
PAD_OFF = 200.0  # out-of-range dst offset for padding edges
EPS = 1e-16

f32 = mybir.dt.float32
i32 = mybir.dt.int32

_CACHE = {}


# ---------------------------------------------------------------------------
# host-side preprocessing
# ---------------------------------------------------------------------------
def _preprocess(edge_index):
    """Build per-core, dst-sorted, block-padded edge arrays.

    Returns (t_blocks, per_core) where t_blocks[b] = tiles for dst block b
    (same for every core) and per_core[c] = dict of [128, T_total] arrays.
    """
    n = N_NODES
    nloc = n // NCORES
    nblk = math.ceil(nloc / P)

    src = np.concatenate([edge_index[0], np.arange(n, dtype=np.int64)])
    dst = np.concatenate([edge_index[1], np.arange(n, dtype=np.int64)])
    src = src.astype(np.int32)
    dst = dst.astype(np.int32)

    core_of = dst // nloc
    # per (core, block) edge counts
    blk_of = (dst % nloc) // P
    counts = np.zeros((NCORES, nblk), np.int64)
    np.add.at(counts, (core_of, blk_of), 1)
    t_blocks = np.maximum(1, np.ceil(counts.max(axis=0) / P).astype(np.int64))
    t_total = int(t_blocks.sum())
    col_off = np.concatenate([[0], np.cumsum(t_blocks)])[:-1]

    per_core = []
    order_all = np.argsort(dst, kind="stable")
    dst_sorted_core = core_of[order_all]
    for c in range(NCORES):
        sel = order_all[dst_sorted_core == c]
        s_c = src[sel]
        d_c = dst[sel] - c * nloc  # local dst, sorted ascending
        b_c = d_c // P

        src_g = np.zeros((t_total, P), np.int32)
        dstoff = np.full((t_total, P), PAD_OFF, np.float32)
        dstloc = np.full((t_total, P), nloc, np.int32)  # pad -> zeroed row

        # place each block's edges into its tile range
        starts = np.searchsorted(b_c, np.arange(nblk))
        ends = np.searchsorted(b_c, np.arange(nblk) + 1)
        for b in range(nblk):
            e0, e1 = starts[b], ends[b]
            cnt = e1 - e0
            tb = int(t_blocks[b])
            assert cnt <= tb * P
            flat0 = col_off[b] * P
            rows = np.arange(cnt)
            src_g.reshape(-1)[flat0 + rows] = s_c[e0:e1]
            dstoff.reshape(-1)[flat0 + rows] = (d_c[e0:e1] - b * P).astype(
                np.float32)
            dstloc.reshape(-1)[flat0 + rows] = d_c[e0:e1]

        per_core.append({
            "src_g": np.ascontiguousarray(src_g.T),     # [128, T_total]
            "dstoff": np.ascontiguousarray(dstoff.T),
            "dstloc": np.ascontiguousarray(dstloc.T),
        })
    return t_blocks, per_core


def _host_consts(inputs):
    """Weight-derived constant tensors fed to every core."""
    c = {}
    for li, (din, dout, h) in enumerate(LAYERS):
        wl = inputs[f"W{li}l"].astype(np.float32)
        wr = inputs[f"W{li}r"].astype(np.float32)
        att = inputs[f"a{li}"].astype(np.float32)
        bias = inputs[f"b{li}"].astype(np.float32)
        ch = dout // h
        a_bd = np.zeros((dout, h), np.float32)
        for hh in range(h):
            a_bd[hh * ch:(hh + 1) * ch, hh] = att[hh]
        c[f"w2_{li}"] = np.ascontiguousarray(
            np.concatenate([wl, wr], axis=1))          # [din, 2*dout]
        c[f"abd_{li}"] = a_bd                           # [dout, h]
        c[f"bias_{li}"] = np.ascontiguousarray(
            np.tile(bias[None, :], (P, 1)))             # [128, dout]
    c["ident"] = np.eye(P, dtype=np.float32)
    c["iota"] = np.tile(np.arange(P, dtype=np.float32)[None, :], (P, 1))
    return c


# ---------------------------------------------------------------------------
# device program
# ---------------------------------------------------------------------------
def _build_program(t_blocks, nloc):
    nblk = len(t_blocks)
    t_total = int(t_blocks.sum())
    col_off = np.concatenate([[0], np.cumsum(t_blocks)])[:-1]
    nlocp = nblk * P  # padded local nodes (12544)
    n = nloc * NCORES

    nc = bacc.Bacc("TRN2", target_bir_lowering=False, debug=False,
                   num_devices=NCORES)

    # ---- I/O ----
    x_local = nc.dram_tensor("x_local", [nlocp, 128], f32, kind="ExternalInput")
    srcg_in = nc.dram_tensor("src_g", [P, t_total], i32, kind="ExternalInput")
    doff_in = nc.dram_tensor("dstoff", [P, t_total], f32, kind="ExternalInput")
    dloc_in = nc.dram_tensor("dstloc", [P, t_total], i32, kind="ExternalInput")
    ident_in = nc.dram_tensor("ident", [P, P], f32, kind="ExternalInput")
    iota_in = nc.dram_tensor("iota", [P, P], f32, kind="ExternalInput")
    w2_in, abd_in, bias_in = {}, {}, {}
    for li, (din, dout, h) in enumerate(LAYERS):
        w2_in[li] = nc.dram_tensor(f"w2_{li}", [din, 2 * dout], f32,
                                   kind="ExternalInput")
        abd_in[li] = nc.dram_tensor(f"abd_{li}", [dout, h], f32,
                                    kind="ExternalInput")
        bias_in[li] = nc.dram_tensor(f"bias_{li}", [P, dout], f32,
                                     kind="ExternalInput")
    out_t = nc.dram_tensor("out", [nloc, 64], f32, kind="ExternalOutput")

    with tile.TileContext(nc) as tc, ExitStack() as ctx:
        cn = ctx.enter_context(tc.tile_pool(name="cn", bufs=1))
        dr = ctx.enter_context(tc.tile_pool(name="dr", bufs=1, space="DRAM"))

        # ---- DRAM working buffers ----
        h_loc = [None] * 4
        h_loc[1] = dr.tile([nlocp, 128], f32, tag="h1", name="h1")
        h_loc[2] = dr.tile([nlocp, 128], f32, tag="h2", name="h2")
        hr_buf = {0: dr.tile([nlocp + P, 128], f32, tag="hr01", name="hr01")}
        hr_buf[1] = hr_buf[0]
        hr_buf[2] = dr.tile([nlocp + P, 64], f32, tag="hr2", name="hr2")
        bounce = {li: dr.tile([nloc, LAYERS[li][1]], f32, tag=f"bnc{li}", name=f"bnc{li}")
                  for li in range(3)}
        hlf = {li: dr.tile([n, LAYERS[li][1]], f32, addr_space="Shared",
                           tag=f"hlf{li}", name=f"hlf{li}")
               for li in range(3)}

        # ---- constants in SBUF ----
        ident = cn.tile([P, P], f32)
        nc.sync.dma_start(out=ident[:], in_=ident_in[:, :])
        iota_row = cn.tile([P, P], f32)
        nc.sync.dma_start(out=iota_row[:], in_=iota_in[:, :])
        srcg = cn.tile([P, t_total], i32)
        nc.sync.dma_start(out=srcg[:], in_=srcg_in[:, :])
        doff = cn.tile([P, t_total], f32)
        nc.sync.dma_start(out=doff[:], in_=doff_in[:, :])
        dloc = cn.tile([P, t_total], i32)
        nc.sync.dma_start(out=dloc[:], in_=dloc_in[:, :])
        w2_sb, abd_sb, bias_sb = {}, {}, {}
        for li, (din, dout, h) in enumerate(LAYERS):
            w2_sb[li] = cn.tile([din, 2 * dout], f32, tag=f"w2s{li}", name=f"w2s{li}")
            nc.sync.dma_start(out=w2_sb[li][:], in_=w2_in[li][:, :])
            abd_sb[li] = cn.tile([dout, h], f32, tag=f"abds{li}", name=f"abds{li}")
            nc.sync.dma_start(out=abd_sb[li][:], in_=abd_in[li][:, :])
            bias_sb[li] = cn.tile([P, dout], f32, tag=f"biass{li}", name=f"biass{li}")
            nc.sync.dma_start(out=bias_sb[li][:], in_=bias_in[li][:, :])

        # zero the pad rows of the hr buffers (rows nloc .. nlocp+P)
        zpad = cn.tile([P, 128], f32)
        nc.vector.memset(zpad[:], 0.0)
        npad = nlocp + P - nloc
        for r0 in range(nloc, nlocp + P, P):
            rows = min(P, nlocp + P - r0)
            nc.sync.dma_start(out=hr_buf[0][r0:r0 + rows, :],
                              in_=zpad[:rows, :])
            nc.sync.dma_start(out=hr_buf[2][r0:r0 + rows, :64],
                              in_=zpad[:rows, :64])

        # ================= layers =================
        for li, (din, dout, h) in enumerate(LAYERS):
            ch = dout // h
            hsrc = x_local if li == 0 else h_loc[li]

            # ---- node phase: HL | HR = h @ [Wl | Wr] ----
            with tc.tile_pool(name=f"nps{li}", bufs=2, space="PSUM") as nps, \
                 tc.tile_pool(name=f"nsb{li}", bufs=3) as nsb:
                for nt in range(nblk):
                    r0 = nt * P
                    rows = min(P, nloc - r0)
                    x_sb = nsb.tile([P, din], f32, tag="x")
                    nc.sync.dma_start(out=x_sb[:], in_=hsrc[r0:r0 + P, :din])
                    xT_ps = nps.tile([P, P], f32, tag="xT")
                    nc.tensor.transpose(out=xT_ps[:, :din], in_=x_sb[:],
                                        identity=ident[:])
                    xT = nsb.tile([P, P], f32, tag="xTs")
                    nc.vector.tensor_copy(out=xT[:din, :], in_=xT_ps[:din, :])
                    hlr_ps = nps.tile([P, 2 * dout], f32, tag="hlr")
                    nc.tensor.matmul(out=hlr_ps[:], lhsT=xT[:din, :],
                                     rhs=w2_sb[li][:], start=True, stop=True)
                    hl_sb = nsb.tile([P, dout], f32, tag="hl")
                    nc.scalar.activation(out=hl_sb[:], in_=hlr_ps[:, 0:dout],
                                         func=mybir.ActivationFunctionType.Copy)
                    hr_sb = nsb.tile([P, dout], f32, tag="hr")
                    nc.scalar.activation(out=hr_sb[:], in_=hlr_ps[:, dout:],
                                         func=mybir.ActivationFunctionType.Copy)
                    nc.sync.dma_start(out=bounce[li][r0:r0 + rows, :],
                                      in_=hl_sb[:rows, :])
                    nc.sync.dma_start(out=hr_buf[li][r0:r0 + rows, :dout],
                                      in_=hr_sb[:rows, :])

            # ---- all-gather HL ----
            nc.gpsimd.collective_compute(
                "AllGather", mybir.AluOpType.bypass,
                replica_groups=[list(range(NCORES))],
                ins=[bounce[li][:].opt()], outs=[hlf[li][:].opt()])

            # ---- edge phase ----
            with tc.tile_pool(name=f"eps{li}", bufs=2, space="PSUM") as eps, \
                 tc.tile_pool(name=f"ewe{li}", bufs=1, space="PSUM") as ewe, \
                 tc.tile_pool(name=f"esb{li}", bufs=3) as esb:
                for b in range(nblk):
                    r0 = b * P
                    rows = min(P, nloc - r0)
                    tb = int(t_blocks[b])
                    u_ps = eps.tile([P, dout + h], f32, tag="U")
                    for t in range(tb):
                        col = int(col_off[b]) + t
                        G = esb.tile([P, dout], f32, tag="G")
                        nc.gpsimd.indirect_dma_start(
                            out=G[:], out_offset=None, in_=hlf[li][:],
                            in_offset=bass.IndirectOffsetOnAxis(
                                ap=srcg[:, col:col + 1], axis=0))
                        R = esb.tile([P, dout], f32, tag="R")
                        nc.gpsimd.indirect_dma_start(
                            out=R[:], out_offset=None,
                            in_=hr_buf[li][:, :dout] if dout != 128
                            else hr_buf[li][:],
                            in_offset=bass.IndirectOffsetOnAxis(
                                ap=dloc[:, col:col + 1], axis=0))
                        S_e = esb.tile([P, P], f32, tag="S")
                        nc.vector.tensor_scalar(
                            out=S_e[:], in0=iota_row[:],
                            scalar1=doff[:, col:col + 1], scalar2=None,
                            op0=mybir.AluOpType.is_equal)
                        tt_ps = eps.tile([P, P], f32, tag="tt")
                        nc.tensor.matmul(out=tt_ps[:dout, :], lhsT=G[:],
                                         rhs=ident[:], start=True, stop=False)
                        nc.tensor.matmul(out=tt_ps[:dout, :], lhsT=R[:],
                                         rhs=ident[:], start=False, stop=True)
                        t2t = esb.tile([P, P], f32, tag="t2t")
                        nc.scalar.activation(
                            out=t2t[:dout, :], in_=tt_ps[:dout, :],
                            func=mybir.ActivationFunctionType.Prelu,
                            alpha=NEG_SLOPE)
                        sc_ps = eps.tile([h, P], f32, tag="sc")
                        nc.tensor.matmul(out=sc_ps[:], lhsT=abd_sb[li][:],
                                         rhs=t2t[:dout, :], start=True,
                                         stop=True)
                        wT = esb.tile([h, P], f32, tag="wT")
                        nc.scalar.activation(
                            out=wT[:], in_=sc_ps[:],
                            func=mybir.ActivationFunctionType.Exp)
                        we_ps = ewe.tile([P, h], f32, tag="wE")
                        nc.tensor.transpose(out=we_ps[:], in_=wT[:],
                                            identity=ident[:h, :h])
                        rhs_seg = esb.tile([P, dout + h], f32, tag="rseg")
                        nc.vector.tensor_copy(out=rhs_seg[:, dout:dout + h],
                                              in_=we_ps[:])
                        if h > 1:
                            nc.vector.tensor_tensor(
                                out=rhs_seg[:, 0:dout].rearrange(
                                    "p (h c) -> p h c", h=h),
                                in0=G[:].rearrange("p (h c) -> p h c", h=h),
                                in1=rhs_seg[:, dout:dout + h].to_broadcast(
                                    [P, h, ch]),
                                op=mybir.AluOpType.mult)
                        else:
                            nc.vector.tensor_scalar(
                                out=rhs_seg[:, 0:dout], in0=G[:],
                                scalar1=rhs_seg[:, dout:dout + 1],
                                scalar2=None, op0=mybir.AluOpType.mult)
                        nc.tensor.matmul(out=u_ps[:], lhsT=S_e[:],
                                         rhs=rhs_seg[:], start=(t == 0),
                                         stop=(t == tb - 1))

                    # ---- block epilogue ----
                    den = esb.tile([P, h], f32, tag="den")
                    nc.vector.tensor_scalar(
                        out=den[:], in0=u_ps[:, dout:dout + h], scalar1=EPS,
                        scalar2=None, op0=mybir.AluOpType.add)
                    rden = esb.tile([P, h], f32, tag="rden")
                    nc.vector.reciprocal(out=rden[:], in_=den[:])
                    o_sb = esb.tile([P, dout], f32, tag="osb")
                    if h > 1:
                        nc.vector.tensor_tensor(
                            out=o_sb[:].rearrange("p (h c) -> p h c", h=h),
                            in0=u_ps[:, 0:dout].rearrange(
                                "p (h c) -> p h c", h=h),
                            in1=rden[:].to_broadcast([P, h, ch]),
                            op=mybir.AluOpType.mult)
                    else:
                        nc.vector.tensor_scalar(
                            out=o_sb[:], in0=u_ps[:, 0:dout],
                            scalar1=rden[:, 0:1], scalar2=None,
                            op0=mybir.AluOpType.mult)
                    nc.vector.tensor_tensor(out=o_sb[:], in0=o_sb[:],
                                            in1=bias_sb[li][:, :dout],
                                            op=mybir.AluOpType.add)
                    o2_sb = esb.tile([P, dout], f32, tag="o2sb")
                    nc.scalar.activation(out=o2_sb[:], in_=o_sb[:],
                                         func=mybir.ActivationFunctionType.Relu)
                    if li < 2:
                        nc.sync.dma_start(
                            out=h_loc[li + 1][r0:r0 + rows, :dout],
                            in_=o2_sb[:rows, :])
                    else:
                        nc.sync.dma_start(out=out_t[r0:r0 + rows, :],
                                          in_=o2_sb[:rows, :])

    nc.compile()
    return nc


def _run(inputs, trace=False):
    n = N_NODES
    nloc = n // NCORES
    nblk = math.ceil(nloc / P)
    nlocp = nblk * P

    key = "prog"
    if key not in _CACHE:
        t_blocks, per_core = _preprocess(np.asarray(inputs["edge_index"]))
        _CACHE["pre"] = (t_blocks, per_core)
        _CACHE[key] = _build_program(t_blocks, nloc)
    nc = _CACHE[key]
    t_blocks, per_core = _CACHE["pre"]

    consts = _host_consts(inputs)
    x = np.asarray(inputs["x"], np.float32)
    in_maps = []
    for c in range(NCORES):
        xl = np.zeros((nlocp, 128), np.float32)
        xl[:nloc] = x[c * nloc:(c + 1) * nloc]
        m = {"x_local": xl, **per_core[c], **consts}
        in_maps.append(m)

    res = run_bass_kernel_spmd(nc, in_maps, core_ids=list(range(NCORES)),
                               trace=trace)
    out = np.concatenate([res.results[c]["out"] for c in range(NCORES)],
                         axis=0)
    return out, res.exec_time_ns


def kernel(**inputs):
    return _run(inputs)[0]


# revision 8
# speedup vs baseline: 2.1442x; 2.1442x over previous
"""GATv2 3-layer encoder on 8 Trainium2 NeuronCores (Bass/Tile).

Sharding: nodes split contiguously across 8 cores (graph parallel). Edges
(with self-loops) are owned by the dst node's core and sorted by dst.
Per layer: local matmuls (HL|HR = h @ [Wl|Wr]), AllGather of the HL shard,
then an edge phase:
  - hl[src] gathered per 128-edge tile via indirect DMA (bf16, the only
    per-edge random access; GpSimd/SWDGE descriptor emission is the
    bottleneck so nothing else runs there),
  - hr[dst] broadcast to edges with a PE matmul against a run-interval
    selector S_eT built on-chip from the sorted dst offsets,
  - (G+R).T accumulated in PSUM by two PE matmuls, LeakyReLU on ScalarE
    (Prelu; same activation-table set as Exp), per-head scores via one PE
    matmul against the block-diagonal attention matrix (batched over
    4-tile groups), segment softmax denominator + weighted feature sum
    fused into one PE segment-sum matmul per tile against an is_equal
    selector S_e, accumulating per 128-dst block in PSUM,
  - per-block epilogue: normalize, bias, relu (f32).

kernel(**inputs) takes FULL inputs, returns the FULL [100000, 64] f32 output.
"""

import math
from contextlib import ExitStack

import numpy as np
import ml_dtypes

import concourse.bass as bass
import concourse.tile as tile
from concourse import bacc, mybir
from concourse.bass_utils import run_bass_kernel_spmd

# -------- problem config (hardcoded; must match reference.setup_inputs) ----
N_NODES = 100_000
N_EDGES = 1_600_000
NCORES = 8
NEG_SLOPE = 0.2
P = 128
LAYERS = [(128, 128, 4), (128, 128, 4), (128, 64, 1)]  # (D_in, D_out, heads)
PAD_OFF = 200.0
EPS = 1e-16
GRP = 4  # tiles per prelu/score/exp batch group

f32 = mybir.dt.float32
bf16 = mybir.dt.bfloat16
i32 = mybir.dt.int32

_CACHE = {}


# ---------------------------------------------------------------------------
# host-side preprocessing
# ---------------------------------------------------------------------------
def _preprocess(edge_index):
    n = N_NODES
    nloc = n // NCORES
    nblk = math.ceil(nloc / P)

    src = np.concatenate([edge_index[0], np.arange(n, dtype=np.int64)]).astype(np.int32)
    dst = np.concatenate([edge_index[1], np.arange(n, dtype=np.int64)]).astype(np.int32)

    core_of = dst // nloc
    blk_of = (dst % nloc) // P
    counts = np.zeros((NCORES, nblk), np.int64)
    np.add.at(counts, (core_of, blk_of), 1)
    t_blocks = np.maximum(1, np.ceil(counts.max(axis=0) / P).astype(np.int64))
    t_total = int(t_blocks.sum())
    col_off = np.concatenate([[0], np.cumsum(t_blocks)])[:-1]

    per_core = []
    order_all = np.argsort(dst, kind="stable")
    dst_sorted_core = core_of[order_all]
    d_arange = np.arange(P)
    for c in range(NCORES):
        sel = order_all[dst_sorted_core == c]
        s_c = src[sel]
        d_c = dst[sel] - c * nloc
        b_c = d_c // P

        src_g = np.zeros((t_total, P), np.int32)
        dstoff = np.full((t_total, P), PAD_OFF, np.float32)

        starts = np.searchsorted(b_c, np.arange(nblk))
        ends = np.searchsorted(b_c, np.arange(nblk) + 1)
        for b in range(nblk):
            e0, e1 = starts[b], ends[b]
            cnt = e1 - e0
            flat0 = col_off[b] * P
            rows = np.arange(cnt)
            src_g.reshape(-1)[flat0 + rows] = s_c[e0:e1]
            dstoff.reshape(-1)[flat0 + rows] = (d_c[e0:e1] - b * P)

        # run intervals per (tile, dst-offset) for the S_eT selector
        dot = dstoff.reshape(t_total, P)
        sc_arr = np.empty((t_total, P), np.float32)
        ec_arr = np.empty((t_total, P), np.float32)
        for t in range(t_total):
            sc_arr[t] = np.searchsorted(dot[t], d_arange)
            ec_arr[t] = np.searchsorted(dot[t], d_arange + 1)

        per_core.append({
            "src_g": np.ascontiguousarray(src_g.T),                 # [128, T]
            "dstoff": np.ascontiguousarray(dot.T),
            "sc_a": np.ascontiguousarray(sc_arr.T),
            "ec_a": np.ascontiguousarray(ec_arr.T),
        })
    return t_blocks, per_core


def _host_consts(inputs):
    c = {}
    for li, (din, dout, h) in enumerate(LAYERS):
        wl = np.asarray(inputs[f"W{li}l"], np.float32)
        wr = np.asarray(inputs[f"W{li}r"], np.float32)
        att = np.asarray(inputs[f"a{li}"], np.float32)
        bias = np.asarray(inputs[f"b{li}"], np.float32)
        ch = dout // h
        a_bd = np.zeros((dout, h), np.float32)
        for hh in range(h):
            a_bd[hh * ch:(hh + 1) * ch, hh] = att[hh]
        c[f"w2_{li}"] = np.concatenate([wl, wr], axis=1).astype(ml_dtypes.bfloat16)
        c[f"abd_{li}"] = a_bd.astype(ml_dtypes.bfloat16)
        c[f"bias_{li}"] = np.ascontiguousarray(np.tile(bias[None, :], (P, 1)))
    c["ident"] = np.eye(P, dtype=ml_dtypes.bfloat16)
    c["iota"] = np.tile(np.arange(P).astype(ml_dtypes.bfloat16)[None, :], (P, 1))
    return c


# ---------------------------------------------------------------------------
# device program
# ---------------------------------------------------------------------------
def _build_program(t_blocks, nloc):
    nblk = len(t_blocks)
    t_total = int(t_blocks.sum())
    col_off = np.concatenate([[0], np.cumsum(t_blocks)])[:-1]
    nlocp = nblk * P
    n = nloc * NCORES

    nc = bacc.Bacc("TRN2", target_bir_lowering=False, debug=False,
                   num_devices=NCORES)

    x_local = nc.dram_tensor("x_local", [nlocp, 128], bf16, kind="ExternalInput")
    srcg_in = nc.dram_tensor("src_g", [P, t_total], i32, kind="ExternalInput")
    doff_in = nc.dram_tensor("dstoff", [P, t_total], f32, kind="ExternalInput")
    sc_in = nc.dram_tensor("sc_a", [P, t_total], f32, kind="ExternalInput")
    ec_in = nc.dram_tensor("ec_a", [P, t_total], f32, kind="ExternalInput")
    ident_in = nc.dram_tensor("ident", [P, P], bf16, kind="ExternalInput")
    iota_in = nc.dram_tensor("iota", [P, P], bf16, kind="ExternalInput")
    w2_in, abd_in, bias_in = {}, {}, {}
    for li, (din, dout, h) in enumerate(LAYERS):
        w2_in[li] = nc.dram_tensor(f"w2_{li}", [din, 2 * dout], bf16,
                                   kind="ExternalInput")
        abd_in[li] = nc.dram_tensor(f"abd_{li}", [dout, h], bf16,
                                    kind="ExternalInput")
        bias_in[li] = nc.dram_tensor(f"bias_{li}", [P, dout], f32,
                                     kind="ExternalInput")
    out_t = nc.dram_tensor("out", [nloc, 64], f32, kind="ExternalOutput")

    with tile.TileContext(nc) as tc, ExitStack() as ctx:
        cn = ctx.enter_context(tc.tile_pool(name="cn", bufs=1))
        dr = ctx.enter_context(tc.tile_pool(name="dr", bufs=1, space="DRAM"))

        h_loc = [None] * 3
        h_loc[1] = dr.tile([nlocp, 128], bf16, tag="h1", name="h1")
        h_loc[2] = dr.tile([nlocp, 128], bf16, tag="h2", name="h2")
        hr_buf = {0: dr.tile([nlocp + P, 128], bf16, tag="hr01", name="hr01")}
        hr_buf[1] = hr_buf[0]
        hr_buf[2] = dr.tile([nlocp + P, 64], bf16, tag="hr2", name="hr2")
        bounce = {li: dr.tile([nloc, LAYERS[li][1]], bf16, tag=f"bnc{li}",
                              name=f"bnc{li}") for li in range(3)}
        hlf = {li: dr.tile([n, LAYERS[li][1]], bf16, addr_space="Shared",
                           tag=f"hlf{li}", name=f"hlf{li}") for li in range(3)}

        ident = cn.tile([P, P], bf16)
        nc.sync.dma_start(out=ident[:], in_=ident_in[:, :])
        iota_row = cn.tile([P, P], bf16)
        nc.sync.dma_start(out=iota_row[:], in_=iota_in[:, :])
        srcg = cn.tile([P, t_total], i32)
        nc.sync.dma_start(out=srcg[:], in_=srcg_in[:, :])
        doff = cn.tile([P, t_total], f32)
        nc.sync.dma_start(out=doff[:], in_=doff_in[:, :])
        sc_t = cn.tile([P, t_total], f32)
        nc.sync.dma_start(out=sc_t[:], in_=sc_in[:, :])
        ec_t = cn.tile([P, t_total], f32)
        nc.sync.dma_start(out=ec_t[:], in_=ec_in[:, :])
        w2_sb, abd_sb, bias_sb = {}, {}, {}
        for li, (din, dout, h) in enumerate(LAYERS):
            w2_sb[li] = cn.tile([din, 2 * dout], bf16, tag=f"w2s{li}",
                                name=f"w2s{li}")
            nc.sync.dma_start(out=w2_sb[li][:], in_=w2_in[li][:, :])
            abd_sb[li] = cn.tile([dout, h], bf16, tag=f"abds{li}",
                                 name=f"abds{li}")
            nc.sync.dma_start(out=abd_sb[li][:], in_=abd_in[li][:, :])
            bias_sb[li] = cn.tile([P, dout], f32, tag=f"biass{li}",
                                  name=f"biass{li}")
            nc.sync.dma_start(out=bias_sb[li][:], in_=bias_in[li][:, :])

        zpad = cn.tile([P, 128], bf16)
        nc.vector.memset(zpad[:], 0.0)
        for r0 in range(nloc, nlocp + P, P):
            rows = min(P, nlocp + P - r0)
            nc.sync.dma_start(out=hr_buf[0][r0:r0 + rows, :], in_=zpad[:rows, :])
            nc.sync.dma_start(out=hr_buf[2][r0:r0 + rows, :64],
                              in_=zpad[:rows, :64])
        if nlocp > nloc:
            nc.sync.dma_start(out=h_loc[1][nloc:nlocp, :],
                              in_=zpad[:nlocp - nloc, :])
            nc.sync.dma_start(out=h_loc[2][nloc:nlocp, :],
                              in_=zpad[:nlocp - nloc, :])

        # ================= layers =================
        for li, (din, dout, h) in enumerate(LAYERS):
            ch = dout // h
            hsrc = x_local if li == 0 else h_loc[li]

            # ---- node phase ----
            with tc.tile_pool(name=f"nps{li}", bufs=2, space="PSUM") as nps, \
                 tc.tile_pool(name=f"nsb{li}", bufs=3) as nsb:
                for nt in range(nblk):
                    r0 = nt * P
                    rows = min(P, nloc - r0)
                    x_sb = nsb.tile([P, din], bf16, tag="x")
                    nc.sync.dma_start(out=x_sb[:], in_=hsrc[r0:r0 + P, :din])
                    xT_ps = nps.tile([P, P], f32, tag="xT")
                    nc.tensor.matmul(out=xT_ps[:din, :], lhsT=x_sb[:],
                                     rhs=ident[:], start=True, stop=True)
                    xT = nsb.tile([P, P], bf16, tag="xTs")
                    nc.vector.tensor_copy(out=xT[:din, :], in_=xT_ps[:din, :])
                    hlr_ps = nps.tile([P, 2 * dout], f32, tag="hlr")
                    nc.tensor.matmul(out=hlr_ps[:], lhsT=xT[:din, :],
                                     rhs=w2_sb[li][:], start=True, stop=True)
                    hl_sb = nsb.tile([P, dout], bf16, tag="hl")
                    nc.scalar.activation(out=hl_sb[:], in_=hlr_ps[:, 0:dout],
                                         func=mybir.ActivationFunctionType.Copy)
                    hr_sb = nsb.tile([P, dout], bf16, tag="hr")
                    nc.scalar.activation(out=hr_sb[:], in_=hlr_ps[:, dout:],
                                         func=mybir.ActivationFunctionType.Copy)
                    nc.sync.dma_start(out=bounce[li][r0:r0 + rows, :],
                                      in_=hl_sb[:rows, :])
                    nc.sync.dma_start(out=hr_buf[li][r0:r0 + rows, :dout],
                                      in_=hr_sb[:rows, :])

            nc.gpsimd.collective_compute(
                "AllGather", mybir.AluOpType.bypass,
                replica_groups=[list(range(NCORES))],
                ins=[bounce[li][:].opt()], outs=[hlf[li][:].opt()])

            # ---- edge phase ----
            with tc.tile_pool(name=f"eps{li}", bufs=2, space="PSUM") as eps, \
                 tc.tile_pool(name=f"ewe{li}", bufs=2, space="PSUM") as ewe, \
                 tc.tile_pool(name=f"esb{li}", bufs=3) as esb:
                for b in range(nblk):
                    r0 = b * P
                    rows = min(P, nloc - r0)
                    tb = int(t_blocks[b])
                    hrb = esb.tile([P, dout], bf16, tag="hrb")
                    nc.sync.dma_start(out=hrb[:],
                                      in_=hr_buf[li][r0:r0 + P, :dout])
                    u_ps = eps.tile([P, dout + h], f32, tag="U")
                    for g0 in range(0, tb, GRP):
                        gts = list(range(g0, min(g0 + GRP, tb)))
                        ncols = len(gts) * P
                        tt_ps = eps.tile([P, GRP * P], f32, tag="tt")
                        Gs, Ss = [], []
                        for gi, t in enumerate(gts):
                            col = int(col_off[b]) + t
                            G = esb.tile([P, dout], bf16, tag=f"G{gi}",
                                         name=f"G{gi}")
                            nc.gpsimd.indirect_dma_start(
                                out=G[:], out_offset=None, in_=hlf[li][:],
                                in_offset=bass.IndirectOffsetOnAxis(
                                    ap=srcg[:, col:col + 1], axis=0))
                            Gs.append(G)
                            S_e = esb.tile([P, P], bf16, tag=f"S{gi}",
                                           name=f"S{gi}")
                            nc.vector.tensor_scalar(
                                out=S_e[:], in0=iota_row[:],
                                scalar1=doff[:, col:col + 1], scalar2=None,
                                op0=mybir.AluOpType.is_equal)
                            Ss.append(S_e)
                            tmp = esb.tile([P, P], bf16, tag="tmp")
                            nc.vector.tensor_scalar(
                                out=tmp[:], in0=iota_row[:],
                                scalar1=ec_t[:, col:col + 1], scalar2=None,
                                op0=mybir.AluOpType.is_lt)
                            S_eT = esb.tile([P, P], bf16, tag=f"ST{gi}",
                                            name=f"ST{gi}")
                            nc.vector.scalar_tensor_tensor(
                                out=S_eT[:], in0=iota_row[:],
                                scalar=sc_t[:, col:col + 1], in1=tmp[:],
                                op0=mybir.AluOpType.is_ge,
                                op1=mybir.AluOpType.mult)
                            cs = slice(gi * P, (gi + 1) * P)
                            nc.tensor.matmul(out=tt_ps[:dout, cs], lhsT=G[:],
                                             rhs=ident[:], start=True,
                                             stop=False)
                            nc.tensor.matmul(out=tt_ps[:dout, cs], lhsT=hrb[:],
                                             rhs=S_eT[:], start=False,
                                             stop=True)
                        t2t = esb.tile([P, GRP * P], bf16, tag="t2t")
                        nc.scalar.activation(
                            out=t2t[:dout, :ncols], in_=tt_ps[:dout, :ncols],
                            func=mybir.ActivationFunctionType.Prelu,
                            alpha=NEG_SLOPE)
                        sc_ps = ewe.tile([h, GRP * P], f32, tag="sc")
                        nc.tensor.matmul(out=sc_ps[:, :ncols],
                                         lhsT=abd_sb[li][:],
                                         rhs=t2t[:dout, :ncols], start=True,
                                         stop=True)
                        wT = esb.tile([h, GRP * P], bf16, tag="wT")
                        nc.scalar.activation(
                            out=wT[:, :ncols], in_=sc_ps[:, :ncols],
                            func=mybir.ActivationFunctionType.Exp)
                        for gi, t in enumerate(gts):
                            cs = slice(gi * P, (gi + 1) * P)
                            we_ps = ewe.tile([P, h], f32, tag="wE")
                            nc.tensor.matmul(out=we_ps[:], lhsT=wT[:, cs],
                                             rhs=ident[:h, :h], start=True,
                                             stop=True)
                            rhs_seg = esb.tile([P, dout + h], bf16, tag="rseg")
                            nc.vector.tensor_copy(
                                out=rhs_seg[:, dout:dout + h], in_=we_ps[:])
                            if h > 1:
                                nc.vector.tensor_tensor(
                                    out=rhs_seg[:, 0:dout].rearrange(
                                        "p (h c) -> p h c", h=h),
                                    in0=Gs[gi][:].rearrange(
                                        "p (h c) -> p h c", h=h),
                                    in1=rhs_seg[:, dout:dout + h].to_broadcast(
                                        [P, h, ch]),
                                    op=mybir.AluOpType.mult)
                            else:
                                nc.vector.tensor_scalar(
                                    out=rhs_seg[:, 0:dout], in0=Gs[gi][:],
                                    scalar1=we_ps[:, 0:1],
                                    scalar2=None, op0=mybir.AluOpType.mult)
                            nc.tensor.matmul(out=u_ps[:], lhsT=Ss[gi][:],
                                             rhs=rhs_seg[:], start=(t == 0),
                                             stop=(t == tb - 1))

                    # ---- block epilogue (f32) ----
                    den = esb.tile([P, h], f32, tag="den")
                    nc.vector.tensor_scalar(
                        out=den[:], in0=u_ps[:, dout:dout + h], scalar1=EPS,
                        scalar2=None, op0=mybir.AluOpType.add)
                    rden = esb.tile([P, h], f32, tag="rden")
                    nc.vector.reciprocal(out=rden[:], in_=den[:])
                    o_sb = esb.tile([P, dout], f32, tag="osb")
                    if h > 1:
                        nc.vector.tensor_tensor(
                            out=o_sb[:].rearrange("p (h c) -> p h c", h=h),
                            in0=u_ps[:, 0:dout].rearrange("p (h c) -> p h c",
                                                          h=h),
                            in1=rden[:].to_broadcast([P, h, ch]),
                            op=mybir.AluOpType.mult)
                    else:
                        nc.vector.tensor_scalar(
                            out=o_sb[:], in0=u_ps[:, 0:dout],
                            scalar1=rden[:, 0:1], scalar2=None,
                            op0=mybir.AluOpType.mult)
                    nc.vector.tensor_tensor(out=o_sb[:], in0=o_sb[:],
                                            in1=bias_sb[li][:, :dout],
                                            op=mybir.AluOpType.add)
                    if li < 2:
                        o2_sb = esb.tile([P, dout], bf16, tag="o2sb")
                        nc.scalar.activation(
                            out=o2_sb[:], in_=o_sb[:],
                            func=mybir.ActivationFunctionType.Relu)
                        nc.sync.dma_start(
                            out=h_loc[li + 1][r0:r0 + rows, :dout],
                            in_=o2_sb[:rows, :])
                    else:
                        o2f = esb.tile([P, dout], f32, tag="o2f")
                        nc.scalar.activation(
                            out=o2f[:], in_=o_sb[:],
                            func=mybir.ActivationFunctionType.Relu)
                        nc.sync.dma_start(out=out_t[r0:r0 + rows, :],
                                          in_=o2f[:rows, :])

    nc.compile()
    return nc


def _run(inputs, trace=False):
    n = N_NODES
    nloc = n // NCORES
    nblk = math.ceil(nloc / P)
    nlocp = nblk * P

    if "prog" not in _CACHE:
        t_blocks, per_core = _preprocess(np.asarray(inputs["edge_index"]))
        _CACHE["pre"] = (t_blocks, per_core)
        _CACHE["prog"] = _build_program(t_blocks, nloc)
    nc = _CACHE["prog"]
    t_blocks, per_core = _CACHE["pre"]

    consts = _host_consts(inputs)
    x = np.asarray(inputs["x"], np.float32)
    in_maps = []
    for c in range(NCORES):
        xl = np.zeros((nlocp, 128), ml_dtypes.bfloat16)
        xl[:nloc] = x[c * nloc:(c + 1) * nloc].astype(ml_dtypes.bfloat16)
        in_maps.append({"x_local": xl, **per_core[c], **consts})

    res = run_bass_kernel_spmd(nc, in_maps, core_ids=list(range(NCORES)),
                               trace=trace)
    out = np.concatenate([res.results[c]["out"] for c in range(NCORES)],
                         axis=0)
    return out, res.exec_time_ns


def kernel(**inputs):
    return _run(inputs)[0]


# revision 9
# speedup vs baseline: 2.1484x; 1.0020x over previous
"""GATv2 3-layer encoder on 8 Trainium2 NeuronCores (Bass/Tile).

Sharding: nodes split contiguously across 8 cores (graph parallel). Edges
(with self-loops) are owned by the dst node's core and sorted by dst.
Per layer: local matmuls (HL|HR = h @ [Wl|Wr]), AllGather of the HL shard,
then an edge phase:
  - hl[src] gathered per 128-edge tile via indirect DMA (bf16, the only
    per-edge random access; GpSimd/SWDGE descriptor emission is the
    bottleneck so nothing else runs there),
  - hr[dst] broadcast to edges with a PE matmul against a run-interval
    selector S_eT built on-chip from the sorted dst offsets,
  - (G+R).T accumulated in PSUM by two PE matmuls, LeakyReLU on ScalarE
    (Prelu; same activation-table set as Exp), per-head scores via one PE
    matmul against the block-diagonal attention matrix (batched over
    4-tile groups), segment softmax denominator + weighted feature sum
    fused into one PE segment-sum matmul per tile against an is_equal
    selector S_e, accumulating per 128-dst block in PSUM,
  - per-block epilogue: normalize, bias, relu (f32).

kernel(**inputs) takes FULL inputs, returns the FULL [100000, 64] f32 output.
"""

import math
from contextlib import ExitStack

import numpy as np
import ml_dtypes

import concourse.bass as bass
import concourse.tile as tile
from concourse import bacc, mybir
from concourse.bass_utils import run_bass_kernel_spmd

# -------- problem config (hardcoded; must match reference.setup_inputs) ----
N_NODES = 100_000
N_EDGES = 1_600_000
NCORES = 8
NEG_SLOPE = 0.2
P = 128
LAYERS = [(128, 128, 4), (128, 128, 4), (128, 64, 1)]  # (D_in, D_out, heads)
PAD_OFF = 200.0
EPS = 1e-16
GRP = 4  # tiles per prelu/score/exp batch group

f32 = mybir.dt.float32
bf16 = mybir.dt.bfloat16
i32 = mybir.dt.int32

_CACHE = {}


# ---------------------------------------------------------------------------
# host-side preprocessing
# ---------------------------------------------------------------------------
def _preprocess(edge_index):
    n = N_NODES
    nloc = n // NCORES
    nblk = math.ceil(nloc / P)

    src = np.concatenate([edge_index[0], np.arange(n, dtype=np.int64)]).astype(np.int32)
    dst = np.concatenate([edge_index[1], np.arange(n, dtype=np.int64)]).astype(np.int32)

    core_of = dst // nloc
    blk_of = (dst % nloc) // P
    counts = np.zeros((NCORES, nblk), np.int64)
    np.add.at(counts, (core_of, blk_of), 1)
    t_blocks = np.maximum(1, np.ceil(counts.max(axis=0) / P).astype(np.int64))
    t_total = int(t_blocks.sum())
    col_off = np.concatenate([[0], np.cumsum(t_blocks)])[:-1]

    per_core = []
    order_all = np.argsort(dst, kind="stable")
    dst_sorted_core = core_of[order_all]
    d_arange = np.arange(P)
    for c in range(NCORES):
        sel = order_all[dst_sorted_core == c]
        s_c = src[sel]
        d_c = dst[sel] - c * nloc
        b_c = d_c // P

        src_g = np.zeros((t_total, P), np.int32)
        dstoff = np.full((t_total, P), PAD_OFF, np.float32)

        starts = np.searchsorted(b_c, np.arange(nblk))
        ends = np.searchsorted(b_c, np.arange(nblk) + 1)
        for b in range(nblk):
            e0, e1 = starts[b], ends[b]
            cnt = e1 - e0
            flat0 = col_off[b] * P
            rows = np.arange(cnt)
            src_g.reshape(-1)[flat0 + rows] = s_c[e0:e1]
            dstoff.reshape(-1)[flat0 + rows] = (d_c[e0:e1] - b * P)

        # run intervals per (tile, dst-offset) for the S_eT selector
        dot = dstoff.reshape(t_total, P)
        sc_arr = np.empty((t_total, P), np.float32)
        ec_arr = np.empty((t_total, P), np.float32)
        for t in range(t_total):
            sc_arr[t] = np.searchsorted(dot[t], d_arange)
            ec_arr[t] = np.searchsorted(dot[t], d_arange + 1)

        per_core.append({
            "src_g": np.ascontiguousarray(src_g.T),                 # [128, T]
            "dstoff": np.ascontiguousarray(dot.T),
            "sc_a": np.ascontiguousarray(sc_arr.T),
            "ec_a": np.ascontiguousarray(ec_arr.T),
        })
    return t_blocks, per_core


def _host_consts(inputs):
    c = {}
    for li, (din, dout, h) in enumerate(LAYERS):
        wl = np.asarray(inputs[f"W{li}l"], np.float32)
        wr = np.asarray(inputs[f"W{li}r"], np.float32)
        att = np.asarray(inputs[f"a{li}"], np.float32)
        bias = np.asarray(inputs[f"b{li}"], np.float32)
        ch = dout // h
        a_bd = np.zeros((dout, h), np.float32)
        for hh in range(h):
            a_bd[hh * ch:(hh + 1) * ch, hh] = att[hh]
        c[f"w2_{li}"] = np.concatenate([wl, wr], axis=1).astype(ml_dtypes.bfloat16)
        c[f"abd_{li}"] = a_bd.astype(ml_dtypes.bfloat16)
        c[f"bias_{li}"] = np.ascontiguousarray(np.tile(bias[None, :], (P, 1)))
    c["ident"] = np.eye(P, dtype=ml_dtypes.bfloat16)
    c["iota"] = np.tile(np.arange(P, dtype=np.float32)[None, :], (P, 1))
    return c


# ---------------------------------------------------------------------------
# device program
# ---------------------------------------------------------------------------
def _build_program(t_blocks, nloc):
    nblk = len(t_blocks)
    t_total = int(t_blocks.sum())
    col_off = np.concatenate([[0], np.cumsum(t_blocks)])[:-1]
    nlocp = nblk * P
    n = nloc * NCORES

    nc = bacc.Bacc("TRN2", target_bir_lowering=False, debug=False,
                   num_devices=NCORES)

    x_local = nc.dram_tensor("x_local", [nlocp, 128], bf16, kind="ExternalInput")
    srcg_in = nc.dram_tensor("src_g", [P, t_total], i32, kind="ExternalInput")
    doff_in = nc.dram_tensor("dstoff", [P, t_total], f32, kind="ExternalInput")
    sc_in = nc.dram_tensor("sc_a", [P, t_total], f32, kind="ExternalInput")
    ec_in = nc.dram_tensor("ec_a", [P, t_total], f32, kind="ExternalInput")
    ident_in = nc.dram_tensor("ident", [P, P], bf16, kind="ExternalInput")
    iota_in = nc.dram_tensor("iota", [P, P], f32, kind="ExternalInput")
    w2_in, abd_in, bias_in = {}, {}, {}
    for li, (din, dout, h) in enumerate(LAYERS):
        w2_in[li] = nc.dram_tensor(f"w2_{li}", [din, 2 * dout], bf16,
                                   kind="ExternalInput")
        abd_in[li] = nc.dram_tensor(f"abd_{li}", [dout, h], bf16,
                                    kind="ExternalInput")
        bias_in[li] = nc.dram_tensor(f"bias_{li}", [P, dout], f32,
                                     kind="ExternalInput")
    out_t = nc.dram_tensor("out", [nloc, 64], f32, kind="ExternalOutput")

    with tile.TileContext(nc) as tc, ExitStack() as ctx:
        cn = ctx.enter_context(tc.tile_pool(name="cn", bufs=1))
        dr = ctx.enter_context(tc.tile_pool(name="dr", bufs=1, space="DRAM"))

        h_loc = [None] * 3
        h_loc[1] = dr.tile([nlocp, 128], bf16, tag="h1", name="h1")
        h_loc[2] = dr.tile([nlocp, 128], bf16, tag="h2", name="h2")
        hr_buf = {0: dr.tile([nlocp + P, 128], bf16, tag="hr01", name="hr01")}
        hr_buf[1] = hr_buf[0]
        hr_buf[2] = dr.tile([nlocp + P, 64], bf16, tag="hr2", name="hr2")
        bounce = {li: dr.tile([nloc, LAYERS[li][1]], bf16, tag=f"bnc{li}",
                              name=f"bnc{li}") for li in range(3)}
        hlf = {li: dr.tile([n, LAYERS[li][1]], bf16, addr_space="Shared",
                           tag=f"hlf{li}", name=f"hlf{li}") for li in range(3)}

        ident = cn.tile([P, P], bf16)
        nc.sync.dma_start(out=ident[:], in_=ident_in[:, :])
        iota_row = cn.tile([P, P], f32)
        nc.sync.dma_start(out=iota_row[:], in_=iota_in[:, :])
        srcg = cn.tile([P, t_total], i32)
        nc.sync.dma_start(out=srcg[:], in_=srcg_in[:, :])
        doff = cn.tile([P, t_total], f32)
        nc.sync.dma_start(out=doff[:], in_=doff_in[:, :])
        sc_t = cn.tile([P, t_total], f32)
        nc.sync.dma_start(out=sc_t[:], in_=sc_in[:, :])
        ec_t = cn.tile([P, t_total], f32)
        nc.sync.dma_start(out=ec_t[:], in_=ec_in[:, :])
        w2_sb, abd_sb, bias_sb = {}, {}, {}
        for li, (din, dout, h) in enumerate(LAYERS):
            w2_sb[li] = cn.tile([din, 2 * dout], bf16, tag=f"w2s{li}",
                                name=f"w2s{li}")
            nc.sync.dma_start(out=w2_sb[li][:], in_=w2_in[li][:, :])
            abd_sb[li] = cn.tile([dout, h], bf16, tag=f"abds{li}",
                                 name=f"abds{li}")
            nc.sync.dma_start(out=abd_sb[li][:], in_=abd_in[li][:, :])
            bias_sb[li] = cn.tile([P, dout], f32, tag=f"biass{li}",
                                  name=f"biass{li}")
            nc.sync.dma_start(out=bias_sb[li][:], in_=bias_in[li][:, :])

        zpad = cn.tile([P, 128], bf16)
        nc.vector.memset(zpad[:], 0.0)
        for r0 in range(nloc, nlocp + P, P):
            rows = min(P, nlocp + P - r0)
            nc.sync.dma_start(out=hr_buf[0][r0:r0 + rows, :], in_=zpad[:rows, :])
            nc.sync.dma_start(out=hr_buf[2][r0:r0 + rows, :64],
                              in_=zpad[:rows, :64])
        if nlocp > nloc:
            nc.sync.dma_start(out=h_loc[1][nloc:nlocp, :],
                              in_=zpad[:nlocp - nloc, :])
            nc.sync.dma_start(out=h_loc[2][nloc:nlocp, :],
                              in_=zpad[:nlocp - nloc, :])

        # ================= layers =================
        for li, (din, dout, h) in enumerate(LAYERS):
            ch = dout // h
            hsrc = x_local if li == 0 else h_loc[li]

            # ---- node phase ----
            with tc.tile_pool(name=f"nps{li}", bufs=2, space="PSUM") as nps, \
                 tc.tile_pool(name=f"nsb{li}", bufs=3) as nsb:
                for nt in range(nblk):
                    r0 = nt * P
                    rows = min(P, nloc - r0)
                    x_sb = nsb.tile([P, din], bf16, tag="x")
                    nc.sync.dma_start(out=x_sb[:], in_=hsrc[r0:r0 + P, :din])
                    xT_ps = nps.tile([P, P], f32, tag="xT")
                    nc.tensor.matmul(out=xT_ps[:din, :], lhsT=x_sb[:],
                                     rhs=ident[:], start=True, stop=True)
                    xT = nsb.tile([P, P], bf16, tag="xTs")
                    nc.vector.tensor_copy(out=xT[:din, :], in_=xT_ps[:din, :])
                    hlr_ps = nps.tile([P, 2 * dout], f32, tag="hlr")
                    nc.tensor.matmul(out=hlr_ps[:], lhsT=xT[:din, :],
                                     rhs=w2_sb[li][:], start=True, stop=True)
                    hl_sb = nsb.tile([P, dout], bf16, tag="hl")
                    nc.scalar.activation(out=hl_sb[:], in_=hlr_ps[:, 0:dout],
                                         func=mybir.ActivationFunctionType.Copy)
                    hr_sb = nsb.tile([P, dout], bf16, tag="hr")
                    nc.scalar.activation(out=hr_sb[:], in_=hlr_ps[:, dout:],
                                         func=mybir.ActivationFunctionType.Copy)
                    nc.sync.dma_start(out=bounce[li][r0:r0 + rows, :],
                                      in_=hl_sb[:rows, :])
                    nc.sync.dma_start(out=hr_buf[li][r0:r0 + rows, :dout],
                                      in_=hr_sb[:rows, :])

            nc.gpsimd.collective_compute(
                "AllGather", mybir.AluOpType.bypass,
                replica_groups=[list(range(NCORES))],
                ins=[bounce[li][:].opt()], outs=[hlf[li][:].opt()])

            # ---- edge phase ----
            with tc.tile_pool(name=f"eps{li}", bufs=2, space="PSUM") as eps, \
                 tc.tile_pool(name=f"ewe{li}", bufs=2, space="PSUM") as ewe, \
                 tc.tile_pool(name=f"esb{li}", bufs=3) as esb:
                for b in range(nblk):
                    r0 = b * P
                    rows = min(P, nloc - r0)
                    tb = int(t_blocks[b])
                    hrb = esb.tile([P, dout], bf16, tag="hrb")
                    nc.sync.dma_start(out=hrb[:],
                                      in_=hr_buf[li][r0:r0 + P, :dout])
                    u_ps = eps.tile([P, dout + h], f32, tag="U")
                    for g0 in range(0, tb, GRP):
                        gts = list(range(g0, min(g0 + GRP, tb)))
                        ncols = len(gts) * P
                        tt_ps = eps.tile([P, GRP * P], f32, tag="tt")
                        Gs, Ss = [], []
                        for gi, t in enumerate(gts):
                            col = int(col_off[b]) + t
                            G = esb.tile([P, dout], bf16, tag=f"G{gi}",
                                         name=f"G{gi}", bufs=5)
                            nc.gpsimd.indirect_dma_start(
                                out=G[:], out_offset=None, in_=hlf[li][:],
                                in_offset=bass.IndirectOffsetOnAxis(
                                    ap=srcg[:, col:col + 1], axis=0))
                            Gs.append(G)
                            S_e = esb.tile([P, P], bf16, tag=f"S{gi}",
                                           name=f"S{gi}")
                            nc.vector.tensor_scalar(
                                out=S_e[:], in0=iota_row[:],
                                scalar1=doff[:, col:col + 1], scalar2=None,
                                op0=mybir.AluOpType.is_equal)
                            Ss.append(S_e)
                            tmp = esb.tile([P, P], bf16, tag="tmp")
                            nc.vector.tensor_scalar(
                                out=tmp[:], in0=iota_row[:],
                                scalar1=ec_t[:, col:col + 1], scalar2=None,
                                op0=mybir.AluOpType.is_lt)
                            S_eT = esb.tile([P, P], bf16, tag=f"ST{gi}",
                                            name=f"ST{gi}")
                            nc.vector.scalar_tensor_tensor(
                                out=S_eT[:], in0=iota_row[:],
                                scalar=sc_t[:, col:col + 1], in1=tmp[:],
                                op0=mybir.AluOpType.is_ge,
                                op1=mybir.AluOpType.mult)
                            cs = slice(gi * P, (gi + 1) * P)
                            nc.tensor.matmul(out=tt_ps[:dout, cs], lhsT=G[:],
                                             rhs=ident[:], start=True,
                                             stop=False)
                            nc.tensor.matmul(out=tt_ps[:dout, cs], lhsT=hrb[:],
                                             rhs=S_eT[:], start=False,
                                             stop=True)
                        t2t = esb.tile([P, GRP * P], bf16, tag="t2t")
                        nc.scalar.activation(
                            out=t2t[:dout, :ncols], in_=tt_ps[:dout, :ncols],
                            func=mybir.ActivationFunctionType.Prelu,
                            alpha=NEG_SLOPE)
                        sc_ps = ewe.tile([h, GRP * P], f32, tag="sc")
                        nc.tensor.matmul(out=sc_ps[:, :ncols],
                                         lhsT=abd_sb[li][:],
                                         rhs=t2t[:dout, :ncols], start=True,
                                         stop=True)
                        wT = esb.tile([h, GRP * P], bf16, tag="wT")
                        nc.scalar.activation(
                            out=wT[:, :ncols], in_=sc_ps[:, :ncols],
                            func=mybir.ActivationFunctionType.Exp)
                        for gi, t in enumerate(gts):
                            cs = slice(gi * P, (gi + 1) * P)
                            we_ps = ewe.tile([P, h], f32, tag="wE")
                            nc.tensor.matmul(out=we_ps[:], lhsT=wT[:, cs],
                                             rhs=ident[:h, :h], start=True,
                                             stop=True)
                            rhs_seg = esb.tile([P, dout + h], bf16, tag="rseg")
                            nc.scalar.activation(
                                out=rhs_seg[:, dout:dout + h], in_=we_ps[:],
                                func=mybir.ActivationFunctionType.Copy)
                            if h > 1:
                                nc.vector.tensor_tensor(
                                    out=rhs_seg[:, 0:dout].rearrange(
                                        "p (h c) -> p h c", h=h),
                                    in0=Gs[gi][:].rearrange(
                                        "p (h c) -> p h c", h=h),
                                    in1=rhs_seg[:, dout:dout + h].to_broadcast(
                                        [P, h, ch]),
                                    op=mybir.AluOpType.mult)
                            else:
                                nc.vector.tensor_scalar(
                                    out=rhs_seg[:, 0:dout], in0=Gs[gi][:],
                                    scalar1=we_ps[:, 0:1],
                                    scalar2=None, op0=mybir.AluOpType.mult)
                            nc.tensor.matmul(out=u_ps[:], lhsT=Ss[gi][:],
                                             rhs=rhs_seg[:], start=(t == 0),
                                             stop=(t == tb - 1))

                    # ---- block epilogue (f32) ----
                    den = esb.tile([P, h], f32, tag="den")
                    nc.vector.tensor_scalar(
                        out=den[:], in0=u_ps[:, dout:dout + h], scalar1=EPS,
                        scalar2=None, op0=mybir.AluOpType.add)
                    rden = esb.tile([P, h], f32, tag="rden")
                    nc.vector.reciprocal(out=rden[:], in_=den[:])
                    o_sb = esb.tile([P, dout], f32, tag="osb")
                    if h > 1:
                        nc.vector.tensor_tensor(
                            out=o_sb[:].rearrange("p (h c) -> p h c", h=h),
                            in0=u_ps[:, 0:dout].rearrange("p (h c) -> p h c",
                                                          h=h),
                            in1=rden[:].to_broadcast([P, h, ch]),
                            op=mybir.AluOpType.mult)
                    else:
                        nc.vector.tensor_scalar(
                            out=o_sb[:], in0=u_ps[:, 0:dout],
                            scalar1=rden[:, 0:1], scalar2=None,
                            op0=mybir.AluOpType.mult)
                    nc.vector.tensor_tensor(out=o_sb[:], in0=o_sb[:],
                                            in1=bias_sb[li][:, :dout],
                                            op=mybir.AluOpType.add)
                    if li < 2:
                        o2_sb = esb.tile([P, dout], bf16, tag="o2sb")
                        nc.scalar.activation(
                            out=o2_sb[:], in_=o_sb[:],
                            func=mybir.ActivationFunctionType.Relu)
                        nc.sync.dma_start(
                            out=h_loc[li + 1][r0:r0 + rows, :dout],
                            in_=o2_sb[:rows, :])
                    else:
                        o2f = esb.tile([P, dout], f32, tag="o2f")
                        nc.scalar.activation(
                            out=o2f[:], in_=o_sb[:],
                            func=mybir.ActivationFunctionType.Relu)
                        nc.sync.dma_start(out=out_t[r0:r0 + rows, :],
                                          in_=o2f[:rows, :])

    nc.compile()
    return nc


def _run(inputs, trace=False):
    n = N_NODES
    nloc = n // NCORES
    nblk = math.ceil(nloc / P)
    nlocp = nblk * P

    if "prog" not in _CACHE:
        t_blocks, per_core = _preprocess(np.asarray(inputs["edge_index"]))
        _CACHE["pre"] = (t_blocks, per_core)
        _CACHE["prog"] = _build_program(t_blocks, nloc)
    nc = _CACHE["prog"]
    t_blocks, per_core = _CACHE["pre"]

    consts = _host_consts(inputs)
    x = np.asarray(inputs["x"], np.float32)
    in_maps = []
    for c in range(NCORES):
        xl = np.zeros((nlocp, 128), ml_dtypes.bfloat16)
        xl[:nloc] = x[c * nloc:(c + 1) * nloc].astype(ml_dtypes.bfloat16)
        in_maps.append({"x_local": xl, **per_core[c], **consts})

    res = run_bass_kernel_spmd(nc, in_maps, core_ids=list(range(NCORES)),
                               trace=trace)
    out = np.concatenate([res.results[c]["out"] for c in range(NCORES)],
                         axis=0)
    return out, res.exec_time_ns


def kernel(**inputs):
    return _run(inputs)[0]


# revision 10
# speedup vs baseline: 2.1608x; 1.0058x over previous
"""GATv2 3-layer encoder on 8 Trainium2 NeuronCores (Bass/Tile).

Sharding: nodes split contiguously across 8 cores (graph parallel). Edges
(with self-loops) are owned by the dst node's core and sorted by dst.
Per layer: local matmuls (HL|HR = h @ [Wl|Wr]), AllGather of the HL shard,
then an edge phase:
  - hl[src] gathered per 128-edge tile via indirect DMA (bf16, the only
    per-edge random access; GpSimd/SWDGE descriptor emission is the
    bottleneck so nothing else runs there),
  - hr[dst] broadcast to edges with a PE matmul against a run-interval
    selector S_eT built on-chip from the sorted dst offsets,
  - (G+R).T accumulated in PSUM by two PE matmuls, LeakyReLU on ScalarE
    (Prelu; same activation-table set as Exp), per-head scores via one PE
    matmul against the block-diagonal attention matrix (batched over
    4-tile groups), segment softmax denominator + weighted feature sum
    fused into one PE segment-sum matmul per tile against an is_equal
    selector S_e, accumulating per 128-dst block in PSUM,
  - per-block epilogue: normalize, bias, relu (f32).

kernel(**inputs) takes FULL inputs, returns the FULL [100000, 64] f32 output.
"""

import math
from contextlib import ExitStack

import numpy as np
import ml_dtypes

import concourse.bass as bass
import concourse.tile as tile
from concourse import bacc, mybir
from concourse.bass_utils import run_bass_kernel_spmd

# -------- problem config (hardcoded; must match reference.setup_inputs) ----
N_NODES = 100_000
N_EDGES = 1_600_000
NCORES = 8
NEG_SLOPE = 0.2
P = 128
LAYERS = [(128, 128, 4), (128, 128, 4), (128, 64, 1)]  # (D_in, D_out, heads)
PAD_OFF = 200.0
EPS = 1e-16
GRP = 4  # tiles per prelu/score/exp batch group

f32 = mybir.dt.float32
bf16 = mybir.dt.bfloat16
i32 = mybir.dt.int32

_CACHE = {}


# ---------------------------------------------------------------------------
# host-side preprocessing
# ---------------------------------------------------------------------------
def _preprocess(edge_index):
    n = N_NODES
    nloc = n // NCORES
    nblk = math.ceil(nloc / P)

    src = np.concatenate([edge_index[0], np.arange(n, dtype=np.int64)]).astype(np.int32)
    dst = np.concatenate([edge_index[1], np.arange(n, dtype=np.int64)]).astype(np.int32)

    core_of = dst // nloc
    blk_of = (dst % nloc) // P
    counts = np.zeros((NCORES, nblk), np.int64)
    np.add.at(counts, (core_of, blk_of), 1)
    t_blocks = np.maximum(1, np.ceil(counts.max(axis=0) / P).astype(np.int64))
    t_total = int(t_blocks.sum())
    col_off = np.concatenate([[0], np.cumsum(t_blocks)])[:-1]

    per_core = []
    order_all = np.argsort(dst, kind="stable")
    dst_sorted_core = core_of[order_all]
    d_arange = np.arange(P)
    for c in range(NCORES):
        sel = order_all[dst_sorted_core == c]
        s_c = src[sel]
        d_c = dst[sel] - c * nloc
        b_c = d_c // P

        src_g = np.zeros((t_total, P), np.int32)
        dstoff = np.full((t_total, P), PAD_OFF, np.float32)

        starts = np.searchsorted(b_c, np.arange(nblk))
        ends = np.searchsorted(b_c, np.arange(nblk) + 1)
        for b in range(nblk):
            e0, e1 = starts[b], ends[b]
            cnt = e1 - e0
            flat0 = col_off[b] * P
            rows = np.arange(cnt)
            src_g.reshape(-1)[flat0 + rows] = s_c[e0:e1]
            dstoff.reshape(-1)[flat0 + rows] = (d_c[e0:e1] - b * P)

        # run intervals per (tile, dst-offset) for the S_eT selector
        dot = dstoff.reshape(t_total, P)
        sc_arr = np.empty((t_total, P), np.float32)
        ec_arr = np.empty((t_total, P), np.float32)
        for t in range(t_total):
            sc_arr[t] = np.searchsorted(dot[t], d_arange)
            ec_arr[t] = np.searchsorted(dot[t], d_arange + 1)

        per_core.append({
            "src_g": np.ascontiguousarray(src_g.T),                 # [128, T]
            "dstoff": np.ascontiguousarray(dot.T),
            "sc_a": np.ascontiguousarray(sc_arr.T),
            "ec_a": np.ascontiguousarray(ec_arr.T),
        })
    return t_blocks, per_core


def _host_consts(inputs):
    c = {}
    for li, (din, dout, h) in enumerate(LAYERS):
        wl = np.asarray(inputs[f"W{li}l"], np.float32)
        wr = np.asarray(inputs[f"W{li}r"], np.float32)
        att = np.asarray(inputs[f"a{li}"], np.float32)
        bias = np.asarray(inputs[f"b{li}"], np.float32)
        ch = dout // h
        a_bd = np.zeros((dout, h), np.float32)
        for hh in range(h):
            a_bd[hh * ch:(hh + 1) * ch, hh] = att[hh]
        c[f"w2_{li}"] = np.concatenate([wl, wr], axis=1).astype(ml_dtypes.bfloat16)
        c[f"abd_{li}"] = a_bd.astype(ml_dtypes.bfloat16)
        c[f"bias_{li}"] = np.ascontiguousarray(np.tile(bias[None, :], (P, 1)))
    c["ident"] = np.eye(P, dtype=ml_dtypes.bfloat16)
    c["iota"] = np.tile(np.arange(P, dtype=np.float32)[None, :], (P, 1))
    return c


# ---------------------------------------------------------------------------
# device program
# ---------------------------------------------------------------------------
def _build_program(t_blocks, nloc):
    nblk = len(t_blocks)
    t_total = int(t_blocks.sum())
    col_off = np.concatenate([[0], np.cumsum(t_blocks)])[:-1]
    nlocp = nblk * P
    n = nloc * NCORES

    nc = bacc.Bacc("TRN2", target_bir_lowering=False, debug=False,
                   num_devices=NCORES)

    x_local = nc.dram_tensor("x_local", [nlocp, 128], bf16, kind="ExternalInput")
    srcg_in = nc.dram_tensor("src_g", [P, t_total], i32, kind="ExternalInput")
    doff_in = nc.dram_tensor("dstoff", [P, t_total], f32, kind="ExternalInput")
    sc_in = nc.dram_tensor("sc_a", [P, t_total], f32, kind="ExternalInput")
    ec_in = nc.dram_tensor("ec_a", [P, t_total], f32, kind="ExternalInput")
    ident_in = nc.dram_tensor("ident", [P, P], bf16, kind="ExternalInput")
    iota_in = nc.dram_tensor("iota", [P, P], f32, kind="ExternalInput")
    w2_in, abd_in, bias_in = {}, {}, {}
    for li, (din, dout, h) in enumerate(LAYERS):
        w2_in[li] = nc.dram_tensor(f"w2_{li}", [din, 2 * dout], bf16,
                                   kind="ExternalInput")
        abd_in[li] = nc.dram_tensor(f"abd_{li}", [dout, h], bf16,
                                    kind="ExternalInput")
        bias_in[li] = nc.dram_tensor(f"bias_{li}", [P, dout], f32,
                                     kind="ExternalInput")
    out_t = nc.dram_tensor("out", [nloc, 64], f32, kind="ExternalOutput")

    with tile.TileContext(nc) as tc, ExitStack() as ctx:
        cn = ctx.enter_context(tc.tile_pool(name="cn", bufs=1))
        dr = ctx.enter_context(tc.tile_pool(name="dr", bufs=1, space="DRAM"))

        h_loc = [None] * 3
        h_loc[1] = dr.tile([nlocp, 128], bf16, tag="h1", name="h1")
        h_loc[2] = dr.tile([nlocp, 128], bf16, tag="h2", name="h2")
        hr_buf = {0: dr.tile([nlocp + P, 128], bf16, tag="hr01", name="hr01")}
        hr_buf[1] = hr_buf[0]
        hr_buf[2] = dr.tile([nlocp + P, 64], bf16, tag="hr2", name="hr2")
        bounce = {li: dr.tile([nloc, LAYERS[li][1]], bf16, tag=f"bnc{li}",
                              name=f"bnc{li}") for li in range(3)}
        hlf = {li: dr.tile([n, LAYERS[li][1]], bf16, addr_space="Shared",
                           tag=f"hlf{li}", name=f"hlf{li}") for li in range(3)}

        ident = cn.tile([P, P], bf16)
        nc.sync.dma_start(out=ident[:], in_=ident_in[:, :])
        iota_row = cn.tile([P, P], f32)
        nc.sync.dma_start(out=iota_row[:], in_=iota_in[:, :])
        srcg = cn.tile([P, t_total], i32)
        nc.sync.dma_start(out=srcg[:], in_=srcg_in[:, :])
        doff = cn.tile([P, t_total], f32)
        nc.sync.dma_start(out=doff[:], in_=doff_in[:, :])
        sc_t = cn.tile([P, t_total], f32)
        nc.sync.dma_start(out=sc_t[:], in_=sc_in[:, :])
        ec_t = cn.tile([P, t_total], f32)
        nc.sync.dma_start(out=ec_t[:], in_=ec_in[:, :])
        w2_sb, abd_sb, bias_sb = {}, {}, {}
        for li, (din, dout, h) in enumerate(LAYERS):
            w2_sb[li] = cn.tile([din, 2 * dout], bf16, tag=f"w2s{li}",
                                name=f"w2s{li}")
            nc.sync.dma_start(out=w2_sb[li][:], in_=w2_in[li][:, :])
            abd_sb[li] = cn.tile([dout, h], bf16, tag=f"abds{li}",
                                 name=f"abds{li}")
            nc.sync.dma_start(out=abd_sb[li][:], in_=abd_in[li][:, :])
            bias_sb[li] = cn.tile([P, dout], f32, tag=f"biass{li}",
                                  name=f"biass{li}")
            nc.sync.dma_start(out=bias_sb[li][:], in_=bias_in[li][:, :])

        zpad = cn.tile([P, 128], bf16)
        nc.vector.memset(zpad[:], 0.0)
        for r0 in range(nloc, nlocp + P, P):
            rows = min(P, nlocp + P - r0)
            nc.sync.dma_start(out=hr_buf[0][r0:r0 + rows, :], in_=zpad[:rows, :])
            nc.sync.dma_start(out=hr_buf[2][r0:r0 + rows, :64],
                              in_=zpad[:rows, :64])
        if nlocp > nloc:
            nc.sync.dma_start(out=h_loc[1][nloc:nlocp, :],
                              in_=zpad[:nlocp - nloc, :])
            nc.sync.dma_start(out=h_loc[2][nloc:nlocp, :],
                              in_=zpad[:nlocp - nloc, :])

        # ================= layers =================
        eps = ctx.enter_context(tc.tile_pool(name="eps", bufs=2, space="PSUM"))
        ewe = ctx.enter_context(tc.tile_pool(name="ewe", bufs=2, space="PSUM"))
        esb = ctx.enter_context(tc.tile_pool(name="esb", bufs=3))
        nsb = ctx.enter_context(tc.tile_pool(name="nsb", bufs=6))
        for li, (din, dout, h) in enumerate(LAYERS):
            ch = dout // h
            hsrc = x_local if li == 0 else h_loc[li]

            # ---- node phase ----
            if True:
                for nt in range(nblk):
                    r0 = nt * P
                    rows = min(P, nloc - r0)
                    x_sb = nsb.tile([P, din], bf16, tag="x")
                    nc.sync.dma_start(out=x_sb[:], in_=hsrc[r0:r0 + P, :din])
                    xT_ps = eps.tile([P, GRP * P], f32, tag="tt")
                    nc.tensor.matmul(out=xT_ps[:din, :P], lhsT=x_sb[:],
                                     rhs=ident[:], start=True, stop=True)
                    xT = nsb.tile([P, P], bf16, tag="xTs")
                    nc.vector.tensor_copy(out=xT[:din, :], in_=xT_ps[:din, :P])
                    hlr_ps = ewe.tile([P, 2 * dout], f32, tag="sc")
                    nc.tensor.matmul(out=hlr_ps[:], lhsT=xT[:din, :],
                                     rhs=w2_sb[li][:], start=True, stop=True)
                    hl_sb = nsb.tile([P, dout], bf16, tag="hl")
                    nc.scalar.activation(out=hl_sb[:], in_=hlr_ps[:, 0:dout],
                                         func=mybir.ActivationFunctionType.Copy)
                    hr_sb = nsb.tile([P, dout], bf16, tag="hr")
                    nc.scalar.activation(out=hr_sb[:], in_=hlr_ps[:, dout:],
                                         func=mybir.ActivationFunctionType.Copy)
                    nc.sync.dma_start(out=bounce[li][r0:r0 + rows, :],
                                      in_=hl_sb[:rows, :])
                    nc.sync.dma_start(out=hr_buf[li][r0:r0 + rows, :dout],
                                      in_=hr_sb[:rows, :])

            nc.gpsimd.collective_compute(
                "AllGather", mybir.AluOpType.bypass,
                replica_groups=[list(range(NCORES))],
                ins=[bounce[li][:].opt()], outs=[hlf[li][:].opt()])

            # ---- edge phase ----
            if True:
                for b in range(nblk):
                    r0 = b * P
                    rows = min(P, nloc - r0)
                    tb = int(t_blocks[b])
                    hrb = esb.tile([P, dout], bf16, tag="hrb")
                    nc.sync.dma_start(out=hrb[:],
                                      in_=hr_buf[li][r0:r0 + P, :dout])
                    u_ps = eps.tile([P, dout + h], f32, tag="U")
                    for g0 in range(0, tb, GRP):
                        gts = list(range(g0, min(g0 + GRP, tb)))
                        ncols = len(gts) * P
                        tt_ps = eps.tile([P, GRP * P], f32, tag="tt")
                        Gs, Ss = [], []
                        for gi, t in enumerate(gts):
                            col = int(col_off[b]) + t
                            G = esb.tile([P, dout], bf16, tag=f"G{gi}",
                                         name=f"G{gi}", bufs=5)
                            nc.gpsimd.indirect_dma_start(
                                out=G[:], out_offset=None, in_=hlf[li][:],
                                in_offset=bass.IndirectOffsetOnAxis(
                                    ap=srcg[:, col:col + 1], axis=0))
                            Gs.append(G)
                            S_e = esb.tile([P, P], bf16, tag=f"S{gi}",
                                           name=f"S{gi}")
                            nc.vector.tensor_scalar(
                                out=S_e[:], in0=iota_row[:],
                                scalar1=doff[:, col:col + 1], scalar2=None,
                                op0=mybir.AluOpType.is_equal)
                            Ss.append(S_e)
                            tmp = esb.tile([P, P], bf16, tag="tmp")
                            nc.vector.tensor_scalar(
                                out=tmp[:], in0=iota_row[:],
                                scalar1=ec_t[:, col:col + 1], scalar2=None,
                                op0=mybir.AluOpType.is_lt)
                            S_eT = esb.tile([P, P], bf16, tag=f"ST{gi}",
                                            name=f"ST{gi}")
                            nc.vector.scalar_tensor_tensor(
                                out=S_eT[:], in0=iota_row[:],
                                scalar=sc_t[:, col:col + 1], in1=tmp[:],
                                op0=mybir.AluOpType.is_ge,
                                op1=mybir.AluOpType.mult)
                            cs = slice(gi * P, (gi + 1) * P)
                            nc.tensor.matmul(out=tt_ps[:dout, cs], lhsT=G[:],
                                             rhs=ident[:], start=True,
                                             stop=False)
                            nc.tensor.matmul(out=tt_ps[:dout, cs], lhsT=hrb[:],
                                             rhs=S_eT[:], start=False,
                                             stop=True)
                        t2t = esb.tile([P, GRP * P], bf16, tag="t2t")
                        nc.scalar.activation(
                            out=t2t[:dout, :ncols], in_=tt_ps[:dout, :ncols],
                            func=mybir.ActivationFunctionType.Prelu,
                            alpha=NEG_SLOPE)
                        sc_ps = ewe.tile([h, GRP * P], f32, tag="sc")
                        nc.tensor.matmul(out=sc_ps[:, :ncols],
                                         lhsT=abd_sb[li][:],
                                         rhs=t2t[:dout, :ncols], start=True,
                                         stop=True)
                        wT = esb.tile([h, GRP * P], bf16, tag="wT")
                        nc.scalar.activation(
                            out=wT[:, :ncols], in_=sc_ps[:, :ncols],
                            func=mybir.ActivationFunctionType.Exp)
                        for gi, t in enumerate(gts):
                            cs = slice(gi * P, (gi + 1) * P)
                            we_ps = ewe.tile([P, h], f32, tag="wE")
                            nc.tensor.matmul(out=we_ps[:], lhsT=wT[:, cs],
                                             rhs=ident[:h, :h], start=True,
                                             stop=True)
                            rhs_seg = esb.tile([P, dout + h], bf16, tag="rseg")
                            nc.scalar.activation(
                                out=rhs_seg[:, dout:dout + h], in_=we_ps[:],
                                func=mybir.ActivationFunctionType.Copy)
                            if h > 1:
                                nc.vector.tensor_tensor(
                                    out=rhs_seg[:, 0:dout].rearrange(
                                        "p (h c) -> p h c", h=h),
                                    in0=Gs[gi][:].rearrange(
                                        "p (h c) -> p h c", h=h),
                                    in1=rhs_seg[:, dout:dout + h].to_broadcast(
                                        [P, h, ch]),
                                    op=mybir.AluOpType.mult)
                            else:
                                nc.vector.tensor_scalar(
                                    out=rhs_seg[:, 0:dout], in0=Gs[gi][:],
                                    scalar1=we_ps[:, 0:1],
                                    scalar2=None, op0=mybir.AluOpType.mult)
                            nc.tensor.matmul(out=u_ps[:], lhsT=Ss[gi][:],
                                             rhs=rhs_seg[:], start=(t == 0),
                                             stop=(t == tb - 1))

                    # ---- block epilogue (f32) ----
                    den = esb.tile([P, h], f32, tag="den")
                    nc.vector.tensor_scalar(
                        out=den[:], in0=u_ps[:, dout:dout + h], scalar1=EPS,
                        scalar2=None, op0=mybir.AluOpType.add)
                    rden = esb.tile([P, h], f32, tag="rden")
                    nc.vector.reciprocal(out=rden[:], in_=den[:])
                    o_sb = esb.tile([P, dout], f32, tag="osb")
                    if h > 1:
                        nc.vector.tensor_tensor(
                            out=o_sb[:].rearrange("p (h c) -> p h c", h=h),
                            in0=u_ps[:, 0:dout].rearrange("p (h c) -> p h c",
                                                          h=h),
                            in1=rden[:].to_broadcast([P, h, ch]),
                            op=mybir.AluOpType.mult)
                    else:
                        nc.vector.tensor_scalar(
                            out=o_sb[:], in0=u_ps[:, 0:dout],
                            scalar1=rden[:, 0:1], scalar2=None,
                            op0=mybir.AluOpType.mult)
                    nc.vector.tensor_tensor(out=o_sb[:], in0=o_sb[:],
                                            in1=bias_sb[li][:, :dout],
                                            op=mybir.AluOpType.add)
                    if li < 2:
                        o2_sb = esb.tile([P, dout], bf16, tag="o2sb")
                        nc.scalar.activation(
                            out=o2_sb[:], in_=o_sb[:],
                            func=mybir.ActivationFunctionType.Relu)
                        nc.sync.dma_start(
                            out=h_loc[li + 1][r0:r0 + rows, :dout],
                            in_=o2_sb[:rows, :])
                    else:
                        o2f = esb.tile([P, dout], f32, tag="o2f")
                        nc.scalar.activation(
                            out=o2f[:], in_=o_sb[:],
                            func=mybir.ActivationFunctionType.Relu)
                        nc.sync.dma_start(out=out_t[r0:r0 + rows, :],
                                          in_=o2f[:rows, :])

    nc.compile()
    return nc


def _run(inputs, trace=False):
    n = N_NODES
    nloc = n // NCORES
    nblk = math.ceil(nloc / P)
    nlocp = nblk * P

    if "prog" not in _CACHE:
        t_blocks, per_core = _preprocess(np.asarray(inputs["edge_index"]))
        _CACHE["pre"] = (t_blocks, per_core)
        _CACHE["prog"] = _build_program(t_blocks, nloc)
    nc = _CACHE["prog"]
    t_blocks, per_core = _CACHE["pre"]

    consts = _host_consts(inputs)
    x = np.asarray(inputs["x"], np.float32)
    in_maps = []
    for c in range(NCORES):
        xl = np.zeros((nlocp, 128), ml_dtypes.bfloat16)
        xl[:nloc] = x[c * nloc:(c + 1) * nloc].astype(ml_dtypes.bfloat16)
        in_maps.append({"x_local": xl, **per_core[c], **consts})

    res = run_bass_kernel_spmd(nc, in_maps, core_ids=list(range(NCORES)),
                               trace=trace)
    out = np.concatenate([res.results[c]["out"] for c in range(NCORES)],
                         axis=0)
    return out, res.exec_time_ns


def kernel(**inputs):
    return _run(inputs)[0]


# revision 12
# speedup vs baseline: 2.3641x; 1.0941x over previous
"""GATv2 3-layer encoder on 8 Trainium2 NeuronCores (Bass/Tile).

Sharding: nodes split contiguously across 8 cores (graph parallel). Edges
(with self-loops) are owned by the dst node's core and sorted by dst.
Per layer: local matmuls (HL|HR = h @ [Wl|Wr]), AllGather of the HL shard,
then an edge phase:
  - hl[src] gathered per 128-edge tile via indirect DMA (bf16, the only
    per-edge random access; GpSimd/SWDGE descriptor emission is the
    bottleneck so nothing else runs there),
  - hr[dst] broadcast to edges with a PE matmul against a run-interval
    selector S_eT built on-chip from the sorted dst offsets,
  - (G+R).T accumulated in PSUM by two PE matmuls, LeakyReLU on ScalarE
    (Prelu; same activation-table set as Exp), per-head scores via one PE
    matmul against the block-diagonal attention matrix (batched over
    4-tile groups), segment softmax denominator + weighted feature sum
    fused into one PE segment-sum matmul per tile against an is_equal
    selector S_e, accumulating per 128-dst block in PSUM,
  - per-block epilogue: normalize, bias, relu (f32).

kernel(**inputs) takes FULL inputs, returns the FULL [100000, 64] f32 output.
"""

import math
from contextlib import ExitStack

import numpy as np
import ml_dtypes

import concourse.bass as bass
import concourse.tile as tile
from concourse import bacc, mybir
from concourse.bass_utils import run_bass_kernel_spmd

# -------- problem config (hardcoded; must match reference.setup_inputs) ----
N_NODES = 100_000
N_EDGES = 1_600_000
NCORES = 8
NEG_SLOPE = 0.2
P = 128
LAYERS = [(128, 128, 4), (128, 128, 4), (128, 64, 1)]  # (D_in, D_out, heads)
PAD_OFF = 200.0
EPS = 1e-16
GRP = 4  # tiles per prelu/score/exp batch group

f32 = mybir.dt.float32
bf16 = mybir.dt.bfloat16
i32 = mybir.dt.int32

_CACHE = {}


# ---------------------------------------------------------------------------
# host-side preprocessing
# ---------------------------------------------------------------------------
def _balance_perm(dst, n, nloc, nblk):
    """Assign nodes to (core, block) bins balancing per-bin edge counts.
    Returns pos_of[old_id] -> new global position."""
    import heapq
    deg = np.bincount(dst, minlength=n).astype(np.int64)
    caps = np.zeros((NCORES, nblk), np.int64)
    caps[:, :] = P
    last = nloc - (nblk - 1) * P
    caps[:, nblk - 1] = last
    order = np.argsort(-deg, kind="stable")
    heap = [(0, c * nblk + b) for c in range(NCORES) for b in range(nblk)]
    heapq.heapify(heap)
    fill = np.zeros(NCORES * nblk, np.int64)
    pos_of = np.empty(n, np.int64)
    spill = []
    for v in order:
        while True:
            w, bin_ = heapq.heappop(heap)
            c, b = divmod(bin_, nblk)
            if fill[bin_] < caps[c, b]:
                break
        s = fill[bin_]
        fill[bin_] += 1
        pos_of[v] = c * nloc + b * P + s
        if fill[bin_] < caps[c, b]:
            heapq.heappush(heap, (w + deg[v], bin_))
    return pos_of


def _preprocess(edge_index):
    n = N_NODES
    nloc = n // NCORES
    nblk = math.ceil(nloc / P)

    src = np.concatenate([edge_index[0], np.arange(n, dtype=np.int64)]).astype(np.int32)
    dst = np.concatenate([edge_index[1], np.arange(n, dtype=np.int64)]).astype(np.int32)

    pos_of = _balance_perm(dst, n, nloc, nblk)
    src = pos_of[src].astype(np.int32)
    dst = pos_of[dst].astype(np.int32)

    core_of = dst // nloc
    blk_of = (dst % nloc) // P
    counts = np.zeros((NCORES, nblk), np.int64)
    np.add.at(counts, (core_of, blk_of), 1)
    t_blocks = np.maximum(1, np.ceil(counts.max(axis=0) / P).astype(np.int64))
    t_total = int(t_blocks.sum())
    col_off = np.concatenate([[0], np.cumsum(t_blocks)])[:-1]

    per_core = []
    order_all = np.argsort(dst, kind="stable")
    dst_sorted_core = core_of[order_all]
    d_arange = np.arange(P)
    for c in range(NCORES):
        sel = order_all[dst_sorted_core == c]
        s_c = src[sel]
        d_c = dst[sel] - c * nloc
        b_c = d_c // P

        src_g = np.zeros((t_total, P), np.int32)
        dstoff = np.full((t_total, P), PAD_OFF, np.float32)

        starts = np.searchsorted(b_c, np.arange(nblk))
        ends = np.searchsorted(b_c, np.arange(nblk) + 1)
        for b in range(nblk):
            e0, e1 = starts[b], ends[b]
            cnt = e1 - e0
            flat0 = col_off[b] * P
            rows = np.arange(cnt)
            src_g.reshape(-1)[flat0 + rows] = s_c[e0:e1]
            dstoff.reshape(-1)[flat0 + rows] = (d_c[e0:e1] - b * P)

        # run intervals per (tile, dst-offset) for the S_eT selector
        dot = dstoff.reshape(t_total, P)
        sc_arr = np.empty((t_total, P), np.float32)
        ec_arr = np.empty((t_total, P), np.float32)
        for t in range(t_total):
            sc_arr[t] = np.searchsorted(dot[t], d_arange)
            ec_arr[t] = np.searchsorted(dot[t], d_arange + 1)

        per_core.append({
            "src_g": np.ascontiguousarray(src_g.T),                 # [128, T]
            "dstoff": np.ascontiguousarray(dot.T),
            "sc_a": np.ascontiguousarray(sc_arr.T),
            "ec_a": np.ascontiguousarray(ec_arr.T),
        })
    return t_blocks, per_core, pos_of


def _host_consts(inputs):
    c = {}
    for li, (din, dout, h) in enumerate(LAYERS):
        wl = np.asarray(inputs[f"W{li}l"], np.float32)
        wr = np.asarray(inputs[f"W{li}r"], np.float32)
        att = np.asarray(inputs[f"a{li}"], np.float32)
        bias = np.asarray(inputs[f"b{li}"], np.float32)
        ch = dout // h
        a_bd = np.zeros((dout, h), np.float32)
        for hh in range(h):
            a_bd[hh * ch:(hh + 1) * ch, hh] = att[hh]
        c[f"w2_{li}"] = np.concatenate([wl, wr], axis=1).astype(ml_dtypes.bfloat16)
        c[f"abd_{li}"] = a_bd.astype(ml_dtypes.bfloat16)
        c[f"bias_{li}"] = np.ascontiguousarray(np.tile(bias[None, :], (P, 1)))
    c["ident"] = np.eye(P, dtype=ml_dtypes.bfloat16)
    c["iota"] = np.tile(np.arange(P, dtype=np.float32)[None, :], (P, 1))
    return c


# ---------------------------------------------------------------------------
# device program
# ---------------------------------------------------------------------------
def _build_program(t_blocks, nloc):
    nblk = len(t_blocks)
    t_total = int(t_blocks.sum())
    col_off = np.concatenate([[0], np.cumsum(t_blocks)])[:-1]
    nlocp = nblk * P
    n = nloc * NCORES

    nc = bacc.Bacc("TRN2", target_bir_lowering=False, debug=False,
                   num_devices=NCORES)

    x_local = nc.dram_tensor("x_local", [nlocp, 128], bf16, kind="ExternalInput")
    srcg_in = nc.dram_tensor("src_g", [P, t_total], i32, kind="ExternalInput")
    doff_in = nc.dram_tensor("dstoff", [P, t_total], f32, kind="ExternalInput")
    sc_in = nc.dram_tensor("sc_a", [P, t_total], f32, kind="ExternalInput")
    ec_in = nc.dram_tensor("ec_a", [P, t_total], f32, kind="ExternalInput")
    ident_in = nc.dram_tensor("ident", [P, P], bf16, kind="ExternalInput")
    iota_in = nc.dram_tensor("iota", [P, P], f32, kind="ExternalInput")
    w2_in, abd_in, bias_in = {}, {}, {}
    for li, (din, dout, h) in enumerate(LAYERS):
        w2_in[li] = nc.dram_tensor(f"w2_{li}", [din, 2 * dout], bf16,
                                   kind="ExternalInput")
        abd_in[li] = nc.dram_tensor(f"abd_{li}", [dout, h], bf16,
                                    kind="ExternalInput")
        bias_in[li] = nc.dram_tensor(f"bias_{li}", [P, dout], f32,
                                     kind="ExternalInput")
    out_t = nc.dram_tensor("out", [nloc, 64], f32, kind="ExternalOutput")

    with tile.TileContext(nc) as tc, ExitStack() as ctx:
        cn = ctx.enter_context(tc.tile_pool(name="cn", bufs=1))
        dr = ctx.enter_context(tc.tile_pool(name="dr", bufs=1, space="DRAM"))

        hr_buf = {0: dr.tile([nlocp + P, 128], bf16, tag="hr0", name="hr0")}
        hr_buf[1] = dr.tile([nlocp + P, 128], bf16, tag="hr1", name="hr1")
        hr_buf[2] = dr.tile([nlocp + P, 64], bf16, tag="hr2", name="hr2")
        bounce = {li: dr.tile([nloc, LAYERS[li][1]], bf16, tag=f"bnc{li}",
                              name=f"bnc{li}") for li in range(3)}
        hlf = {li: dr.tile([n, LAYERS[li][1]], bf16, addr_space="Shared",
                           tag=f"hlf{li}", name=f"hlf{li}") for li in range(3)}

        ident = cn.tile([P, P], bf16)
        nc.sync.dma_start(out=ident[:], in_=ident_in[:, :])
        iota_row = cn.tile([P, P], f32)
        nc.sync.dma_start(out=iota_row[:], in_=iota_in[:, :])
        srcg = cn.tile([P, t_total], i32)
        nc.sync.dma_start(out=srcg[:], in_=srcg_in[:, :])
        doff = cn.tile([P, t_total], f32)
        nc.sync.dma_start(out=doff[:], in_=doff_in[:, :])
        sc_t = cn.tile([P, t_total], f32)
        nc.sync.dma_start(out=sc_t[:], in_=sc_in[:, :])
        ec_t = cn.tile([P, t_total], f32)
        nc.sync.dma_start(out=ec_t[:], in_=ec_in[:, :])
        w2_sb, abd_sb, bias_sb = {}, {}, {}
        for li, (din, dout, h) in enumerate(LAYERS):
            w2_sb[li] = cn.tile([din, 2 * dout], bf16, tag=f"w2s{li}",
                                name=f"w2s{li}")
            nc.sync.dma_start(out=w2_sb[li][:], in_=w2_in[li][:, :])
            abd_sb[li] = cn.tile([dout, h], bf16, tag=f"abds{li}",
                                 name=f"abds{li}")
            nc.sync.dma_start(out=abd_sb[li][:], in_=abd_in[li][:, :])
            bias_sb[li] = cn.tile([P, dout], f32, tag=f"biass{li}",
                                  name=f"biass{li}")
            nc.sync.dma_start(out=bias_sb[li][:], in_=bias_in[li][:, :])

        zpad = cn.tile([P, 128], bf16)
        nc.vector.memset(zpad[:], 0.0)
        for r0 in range(nloc, nlocp + P, P):
            rows = min(P, nlocp + P - r0)
            nc.sync.dma_start(out=hr_buf[0][r0:r0 + rows, :], in_=zpad[:rows, :])
            nc.sync.dma_start(out=hr_buf[1][r0:r0 + rows, :], in_=zpad[:rows, :])
            nc.sync.dma_start(out=hr_buf[2][r0:r0 + rows, :64],
                              in_=zpad[:rows, :64])

        # ================= layers =================
        eps = ctx.enter_context(tc.tile_pool(name="eps", bufs=2, space="PSUM"))
        ewe = ctx.enter_context(tc.tile_pool(name="ewe", bufs=2, space="PSUM"))
        esb = ctx.enter_context(tc.tile_pool(name="esb", bufs=3))
        nsb = ctx.enter_context(tc.tile_pool(name="nsb", bufs=6))
        for li, (din, dout, h) in enumerate(LAYERS):
            ch = dout // h
            hsrc = x_local

            # ---- node phase (standalone for layer 0 only; later layers are
            # fused into the previous edge phase epilogue) ----
            if li == 0:
                for nt in range(nblk):
                    r0 = nt * P
                    rows = min(P, nloc - r0)
                    x_sb = nsb.tile([P, din], bf16, tag="x")
                    nc.sync.dma_start(out=x_sb[:], in_=hsrc[r0:r0 + P, :din])
                    xT_ps = eps.tile([P, GRP * P], f32, tag="tt")
                    nc.tensor.matmul(out=xT_ps[:din, :P], lhsT=x_sb[:],
                                     rhs=ident[:], start=True, stop=True)
                    xT = nsb.tile([P, P], bf16, tag="xTs")
                    nc.vector.tensor_copy(out=xT[:din, :], in_=xT_ps[:din, :P])
                    hlr_ps = ewe.tile([P, 2 * dout], f32, tag="sc")
                    nc.tensor.matmul(out=hlr_ps[:], lhsT=xT[:din, :],
                                     rhs=w2_sb[li][:], start=True, stop=True)
                    hl_sb = nsb.tile([P, dout], bf16, tag="hl")
                    nc.scalar.activation(out=hl_sb[:], in_=hlr_ps[:, 0:dout],
                                         func=mybir.ActivationFunctionType.Copy)
                    hr_sb = nsb.tile([P, dout], bf16, tag="hr")
                    nc.scalar.activation(out=hr_sb[:], in_=hlr_ps[:, dout:],
                                         func=mybir.ActivationFunctionType.Copy)
                    nc.sync.dma_start(out=bounce[li][r0:r0 + rows, :],
                                      in_=hl_sb[:rows, :])
                    nc.sync.dma_start(out=hr_buf[li][r0:r0 + rows, :dout],
                                      in_=hr_sb[:rows, :])

            if li == 0:
                nc.gpsimd.collective_compute(
                    "AllGather", mybir.AluOpType.bypass,
                    replica_groups=[list(range(NCORES))],
                    ins=[bounce[0][:].opt()], outs=[hlf[0][:].opt()])

            # ---- edge phase ----
            if True:
                for b in range(nblk):
                    r0 = b * P
                    rows = min(P, nloc - r0)
                    tb = int(t_blocks[b])
                    hrb = esb.tile([P, dout], bf16, tag="hrb")
                    nc.sync.dma_start(out=hrb[:],
                                      in_=hr_buf[li][r0:r0 + P, :dout])
                    u_ps = eps.tile([P, dout + h], f32, tag="U")
                    for g0 in range(0, tb, GRP):
                        gts = list(range(g0, min(g0 + GRP, tb)))
                        ncols = len(gts) * P
                        tt_ps = eps.tile([P, GRP * P], f32, tag="tt")
                        Gs, Ss = [], []
                        for gi, t in enumerate(gts):
                            col = int(col_off[b]) + t
                            G = esb.tile([P, dout], bf16, tag=f"G{gi}",
                                         name=f"G{gi}", bufs=5)
                            nc.gpsimd.indirect_dma_start(
                                out=G[:], out_offset=None, in_=hlf[li][:],
                                in_offset=bass.IndirectOffsetOnAxis(
                                    ap=srcg[:, col:col + 1], axis=0))
                            Gs.append(G)
                            S_e = esb.tile([P, P], bf16, tag=f"S{gi}",
                                           name=f"S{gi}")
                            nc.vector.tensor_scalar(
                                out=S_e[:], in0=iota_row[:],
                                scalar1=doff[:, col:col + 1], scalar2=None,
                                op0=mybir.AluOpType.is_equal)
                            Ss.append(S_e)
                            tmp = esb.tile([P, P], bf16, tag="tmp")
                            nc.vector.tensor_scalar(
                                out=tmp[:], in0=iota_row[:],
                                scalar1=ec_t[:, col:col + 1], scalar2=None,
                                op0=mybir.AluOpType.is_lt)
                            S_eT = esb.tile([P, P], bf16, tag=f"ST{gi}",
                                            name=f"ST{gi}")
                            nc.vector.scalar_tensor_tensor(
                                out=S_eT[:], in0=iota_row[:],
                                scalar=sc_t[:, col:col + 1], in1=tmp[:],
                                op0=mybir.AluOpType.is_ge,
                                op1=mybir.AluOpType.mult)
                            cs = slice(gi * P, (gi + 1) * P)
                            nc.tensor.matmul(out=tt_ps[:dout, cs], lhsT=G[:],
                                             rhs=ident[:], start=True,
                                             stop=False)
                            nc.tensor.matmul(out=tt_ps[:dout, cs], lhsT=hrb[:],
                                             rhs=S_eT[:], start=False,
                                             stop=True)
                        t2t = esb.tile([P, GRP * P], bf16, tag="t2t")
                        nc.scalar.activation(
                            out=t2t[:dout, :ncols], in_=tt_ps[:dout, :ncols],
                            func=mybir.ActivationFunctionType.Prelu,
                            alpha=NEG_SLOPE)
                        sc_ps = ewe.tile([h, GRP * P], f32, tag="sc")
                        nc.tensor.matmul(out=sc_ps[:, :ncols],
                                         lhsT=abd_sb[li][:],
                                         rhs=t2t[:dout, :ncols], start=True,
                                         stop=True)
                        wT = esb.tile([h, GRP * P], bf16, tag="wT")
                        nc.scalar.activation(
                            out=wT[:, :ncols], in_=sc_ps[:, :ncols],
                            func=mybir.ActivationFunctionType.Exp)
                        for gi, t in enumerate(gts):
                            cs = slice(gi * P, (gi + 1) * P)
                            we_ps = ewe.tile([P, h], f32, tag="wE")
                            nc.tensor.matmul(out=we_ps[:], lhsT=wT[:, cs],
                                             rhs=ident[:h, :h], start=True,
                                             stop=True)
                            rhs_seg = esb.tile([P, dout + h], bf16, tag="rseg")
                            nc.scalar.activation(
                                out=rhs_seg[:, dout:dout + h], in_=we_ps[:],
                                func=mybir.ActivationFunctionType.Copy)
                            if h > 1:
                                nc.vector.tensor_tensor(
                                    out=rhs_seg[:, 0:dout].rearrange(
                                        "p (h c) -> p h c", h=h),
                                    in0=Gs[gi][:].rearrange(
                                        "p (h c) -> p h c", h=h),
                                    in1=rhs_seg[:, dout:dout + h].to_broadcast(
                                        [P, h, ch]),
                                    op=mybir.AluOpType.mult)
                            else:
                                nc.vector.tensor_scalar(
                                    out=rhs_seg[:, 0:dout], in0=Gs[gi][:],
                                    scalar1=we_ps[:, 0:1],
                                    scalar2=None, op0=mybir.AluOpType.mult)
                            nc.tensor.matmul(out=u_ps[:], lhsT=Ss[gi][:],
                                             rhs=rhs_seg[:], start=(t == 0),
                                             stop=(t == tb - 1))

                    # ---- block epilogue (f32) ----
                    den = esb.tile([P, h], f32, tag="den")
                    nc.vector.tensor_scalar(
                        out=den[:], in0=u_ps[:, dout:dout + h], scalar1=EPS,
                        scalar2=None, op0=mybir.AluOpType.add)
                    rden = esb.tile([P, h], f32, tag="rden")
                    nc.vector.reciprocal(out=rden[:], in_=den[:])
                    o_sb = esb.tile([P, dout], f32, tag="osb")
                    if h > 1:
                        nc.vector.tensor_tensor(
                            out=o_sb[:].rearrange("p (h c) -> p h c", h=h),
                            in0=u_ps[:, 0:dout].rearrange("p (h c) -> p h c",
                                                          h=h),
                            in1=rden[:].to_broadcast([P, h, ch]),
                            op=mybir.AluOpType.mult)
                    else:
                        nc.vector.tensor_scalar(
                            out=o_sb[:], in0=u_ps[:, 0:dout],
                            scalar1=rden[:, 0:1], scalar2=None,
                            op0=mybir.AluOpType.mult)
                    nc.vector.tensor_tensor(out=o_sb[:], in0=o_sb[:],
                                            in1=bias_sb[li][:, :dout],
                                            op=mybir.AluOpType.add)
                    if li < 2:
                        o2_sb = esb.tile([P, dout], bf16, tag="o2sb")
                        nc.scalar.activation(
                            out=o2_sb[:], in_=o_sb[:],
                            func=mybir.ActivationFunctionType.Relu)
                        # fused node phase for layer li+1, block b
                        din2 = LAYERS[li + 1][0]
                        dout2 = LAYERS[li + 1][1]
                        xT_ps = ewe.tile([P, P], f32, tag="wE", name="xTf")
                        nc.tensor.matmul(out=xT_ps[:din2, :], lhsT=o2_sb[:],
                                         rhs=ident[:], start=True, stop=True)
                        xTf = nsb.tile([P, P], bf16, tag="xTfs", name="xTfs")
                        nc.vector.tensor_copy(out=xTf[:din2, :],
                                              in_=xT_ps[:din2, :])
                        hlr2 = eps.tile([P, 2 * dout2], f32, tag="U",
                                        name="hlr2")
                        nc.tensor.matmul(out=hlr2[:], lhsT=xTf[:din2, :],
                                         rhs=w2_sb[li + 1][:], start=True,
                                         stop=True)
                        hl2 = nsb.tile([P, dout2], bf16, tag="hl2", name="hl2")
                        nc.scalar.activation(
                            out=hl2[:], in_=hlr2[:, 0:dout2],
                            func=mybir.ActivationFunctionType.Copy)
                        hr2s = nsb.tile([P, dout2], bf16, tag="hr2s",
                                        name="hr2s")
                        nc.scalar.activation(
                            out=hr2s[:], in_=hlr2[:, dout2:],
                            func=mybir.ActivationFunctionType.Copy)
                        nc.sync.dma_start(out=bounce[li + 1][r0:r0 + rows, :],
                                          in_=hl2[:rows, :])
                        nc.sync.dma_start(
                            out=hr_buf[li + 1][r0:r0 + rows, :dout2],
                            in_=hr2s[:rows, :])
                    else:
                        o2f = esb.tile([P, dout], f32, tag="o2f")
                        nc.scalar.activation(
                            out=o2f[:], in_=o_sb[:],
                            func=mybir.ActivationFunctionType.Relu)
                        nc.sync.dma_start(out=out_t[r0:r0 + rows, :],
                                          in_=o2f[:rows, :])

                if li < 2:
                    nc.gpsimd.collective_compute(
                        "AllGather", mybir.AluOpType.bypass,
                        replica_groups=[list(range(NCORES))],
                        ins=[bounce[li + 1][:].opt()],
                        outs=[hlf[li + 1][:].opt()])

    nc.compile()
    return nc


def _run(inputs, trace=False):
    n = N_NODES
    nloc = n // NCORES
    nblk = math.ceil(nloc / P)
    nlocp = nblk * P

    if "prog" not in _CACHE:
        t_blocks, per_core, pos_of = _preprocess(np.asarray(inputs["edge_index"]))
        _CACHE["pre"] = (t_blocks, per_core, pos_of)
        _CACHE["prog"] = _build_program(t_blocks, nloc)
    nc = _CACHE["prog"]
    t_blocks, per_core, pos_of = _CACHE["pre"]

    consts = _host_consts(inputs)
    x = np.asarray(inputs["x"], np.float32)
    xp = np.empty_like(x)
    xp[pos_of] = x  # xp[new_pos] = x[old]
    in_maps = []
    for c in range(NCORES):
        xl = np.zeros((nlocp, 128), ml_dtypes.bfloat16)
        xl[:nloc] = xp[c * nloc:(c + 1) * nloc].astype(ml_dtypes.bfloat16)
        in_maps.append({"x_local": xl, **per_core[c], **consts})

    res = run_bass_kernel_spmd(nc, in_maps, core_ids=list(range(NCORES)),
                               trace=trace)
    out = np.concatenate([res.results[c]["out"] for c in range(NCORES)],
                         axis=0)
    out = out[pos_of]  # out_full[old] = out_new[pos_of[old]]
    return out, res.exec_time_ns


def kernel(**inputs):
    return _run(inputs)[0]
